# revision 23
# baseline (speedup 1.0000x reference)
"""CrossAttentionNoProj Trainium2 kernel.

Sharding (8 cores): 2-way data-parallel over batch x 4-way head-parallel
(4 heads / 256 inner dims per core). to_q column-parallel, to_out
row-parallel; per-core partial outputs are summed on gather.

Device algorithm per core (b, head-group g):
  - LayerNorm folded into the q-projection: stats (mean / E[x^2]) via
    ones-matmuls over x^T, q^T = rstd * (Wq'^T @ x^T - wWq outer mu) + bWq
    where Wq' = diag(ln_w) Wq, all computed in q-transposed layout so
    every matmul contracts along partitions.
  - Scores computed transposed (keys on partitions, queries on free axis):
    simT[k, m] = kvT . qT, one 128-row key tile per matmul, two heads
    sharing one 2-bank PSUM tile.
  - exp with the softmax scale folded into the ACT affine; mask applied
    multiplicatively after exp (equivalent since masked exp terms are
    exactly zeroed); kv-append-q handled by using on-chip q as the first
    4 key tiles (mask-free).
  - V-matmul in natural layout with a ones-column appended to kv so the
    softmax denominator falls out as row 64 of the accumulator.
  - Per-head division at aT evacuation, then the row-parallel output
    projection o_part = aT^T @ Wo[g-slice].
"""

import os
import numpy as np

# NTFF trace hooks are unavailable in this container; a stray BASS_TRACE
# would crash the run path, so pin it off unless the caller insists.
os.environ.setdefault("BASS_NEVER_TRACE", "1")

B, NX, NCX, D, H = 2, 512, 4096, 1024, 16
HD = 64
HPC = 4            # heads per core
DL = HPC * HD      # local inner dim = 256
SCALE = HD ** -0.5
LN_EPS = 1e-5
NKQ = NX // 128    # 4 key tiles from appended q
NKC = NCX // 128   # 32 key tiles from context
NKT = NKQ + NKC

_PROG = None
LAST_RESULT = None


def _build_program():
    from concourse import bacc
    import concourse.mybir as mybir
    from concourse.tile import TileContext
    from concourse.masks import make_identity

    F32 = mybir.dt.float32
    F32R = mybir.dt.float32r
    F16 = mybir.dt.float16
    U8 = mybir.dt.uint8

    def r(ap):
        return ap.bitcast(F32R)
    Exp = mybir.ActivationFunctionType.Exp
    Log = mybir.ActivationFunctionType.Ln

    nc = bacc.Bacc(None, target_bir_lowering=False, debug=False)

    xT_in = nc.dram_tensor("xT", (D, NX), F32R, kind="ExternalInput")
    cT_in = nc.dram_tensor("cT", (DL, NCX), F32R, kind="ExternalInput")
    cN_in = nc.dram_tensor("cN", (128, NKC * HPC * (HD + 1)), F16, kind="ExternalInput")
    mk_in = nc.dram_tensor("mk", (NCX, NX), U8, kind="ExternalInput")
    wq_in = nc.dram_tensor("wq", (D, DL), F32R, kind="ExternalInput")
    wo_in = nc.dram_tensor("wo", (DL, D), F32R, kind="ExternalInput")
    lnw_in = nc.dram_tensor("lnw", (D,), F32, kind="ExternalInput")
    lnb_in = nc.dram_tensor("lnb", (D,), F32R, kind="ExternalInput")
    o_out = nc.dram_tensor("o", (NX, D), F32, kind="ExternalOutput")

    with TileContext(nc) as tc:
        with (
            tc.tile_pool(name="cst", bufs=1) as cst,
            tc.tile_pool(name="xsq", bufs=2) as xsqp,
            tc.tile_pool(name="mskf", bufs=3) as mskp,
            tc.tile_pool(name="pt", bufs=8) as ptp,
            tc.tile_pool(name="rb", bufs=2) as rbp,
            tc.tile_pool(name="osb", bufs=4) as osbp,
            tc.tile_pool(name="psA", bufs=2, space="PSUM") as psA,
            tc.tile_pool(name="psB", bufs=4, space="PSUM") as psB,
        ):
            ident = cst.tile([128, 128], F32)
            make_identity(nc, ident)
            ones_col = cst.tile([128, 1], F32R)
            nc.vector.memset(ones_col.bitcast(mybir.dt.uint32), 0x3F800000)
            ones_row = cst.tile([1, 128], F32)
            nc.vector.memset(ones_row, 1.0)

            lnw_t = cst.tile([128, 8], F32)
            nc.sync.dma_start(out=lnw_t, in_=lnw_in[:].rearrange("(t p) -> p t", p=128))
            lnb_t = cst.tile([128, 8], F32R)
            nc.sync.dma_start(out=lnb_t, in_=lnb_in[:].rearrange("(t p) -> p t", p=128))
            xT_t = cst.tile([128, 8, NX], F32R)
            xT_re = xT_in[:].rearrange("(t p) n -> p t n", p=128)
            for t in range(0, 8, 4):
                nc.sync.dma_start(out=xT_t[:, t:t + 4], in_=xT_re[:, t:t + 4])
            wq_t = cst.tile([128, 8, DL], F32R)
            wq_re = wq_in[:].rearrange("(t p) n -> p t n", p=128)
            for t in range(0, 8, 4):
                nc.sync.dma_start(out=wq_t[:, t:t + 4], in_=wq_re[:, t:t + 4])
            cT_t = cst.tile([128, 2, NCX], F32R)
            cT_re = cT_in[:].rearrange("(t p) n -> p t n", p=128)
            mk_t = cst.tile([128, NKC, NX], U8)
            mk_re = mk_in[:].rearrange("(t p) n -> p t n", p=128)
            kvN = cst.tile([128, NKC, HPC, HD + 1], F16)
            # first score chunk + first mask chunk, then the whole kv (one
            # flat DMA — ones column pre-padded on host), then the rest
            nc.sync.dma_start(out=cT_t[:, :, 0:512], in_=cT_re[:, :, 0:512])
            nc.sync.dma_start(out=mk_t[:, 0:16], in_=mk_re[:, 0:16])
            nc.sync.dma_start(
                out=kvN.rearrange("p t h c -> p (t h c)"), in_=cN_in[:]
            )
            for ch in range(1, 8):
                nc.sync.dma_start(
                    out=cT_t[:, :, ch * 512:(ch + 1) * 512],
                    in_=cT_re[:, :, ch * 512:(ch + 1) * 512],
                )
            nc.sync.dma_start(out=mk_t[:, 16:32], in_=mk_re[:, 16:32])
            # wo is only needed for the tail projection — load it last
            wo_t = cst.tile([128, 2, D], F32R)
            wo_re = wo_in[:].rearrange("(t p) n -> p t n", p=128)
            for t in range(2):
                nc.sync.dma_start(out=wo_t[:, t], in_=wo_re[:, t])

            # ---- LayerNorm stats from x^T: mean and E[x^2] per token ----
            ps_su = psA.tile([1, NX], F32, tag="A")
            ps_sq = psA.tile([1, NX], F32, tag="A")
            for t in range(8):
                xsq = xsqp.tile([128, NX], F32R)
                nc.vector.tensor_mul(xsq, xT_t[:, t], xT_t[:, t])
                nc.tensor.matmul(ps_su, ones_col, xT_t[:, t], start=(t == 0), stop=(t == 7))
                nc.tensor.matmul(ps_sq, ones_col, xsq, start=(t == 0), stop=(t == 7))
            mu = cst.tile([1, NX], F32)
            nc.scalar.mul(out=mu, in_=ps_su, mul=1.0 / D)
            ex2 = cst.tile([1, NX], F32)
            nc.scalar.mul(out=ex2, in_=ps_sq, mul=1.0 / D)
            var = cst.tile([1, NX], F32)
            nc.vector.tensor_mul(var, mu, mu)
            nc.vector.tensor_sub(var, ex2, var)
            eps_t = cst.tile([1, 1], F32)
            nc.vector.memset(eps_t, LN_EPS)
            lnv = cst.tile([1, NX], F32)
            nc.scalar.activation(out=lnv, in_=var, func=Log, bias=eps_t)
            rstd = cst.tile([1, NX], F32)
            nc.scalar.activation(out=rstd, in_=lnv, func=Exp, scale=-0.5)
            negmu = cst.tile([1, NX], F32)
            nc.vector.tensor_scalar_mul(negmu, mu, -1.0)
            ps_rb = psA.tile([128, NX], F32, tag="A")
            nc.tensor.matmul(ps_rb, ones_row, rstd, start=True, stop=True)
            rstd_bc = cst.tile([128, NX], F32R)
            nc.vector.tensor_copy(out=rstd_bc, in_=ps_rb)

            # ---- bWq = ln_b @ Wq (raw Wq), then fold ln_w into Wq ----
            ps_bw = psA.tile([128, 2], F32, tag="A")
            for dt in range(2):
                for t in range(8):
                    nc.tensor.matmul(
                        ps_bw[:, dt:dt + 1],
                        wq_t[:, t, dt * 128:(dt + 1) * 128].bitcast(F32),
                        lnb_t[:, t:t + 1].bitcast(F32),
                        start=(t == 0), stop=(t == 7),
                    )
            bwq = cst.tile([128, 2], F32)
            nc.vector.tensor_copy(out=bwq, in_=ps_bw)
            for t in range(8):
                nc.vector.tensor_scalar_mul(wq_t[:, t], wq_t[:, t], lnw_t[:, t:t + 1])
            # wWq[d] = sum_k Wq'[k, d]
            ps_ww = psA.tile([1, DL], F32, tag="A")
            for t in range(8):
                nc.tensor.matmul(ps_ww, ones_col, wq_t[:, t], start=(t == 0), stop=(t == 7))
            wwq = cst.tile([1, DL], F32)
            nc.vector.tensor_copy(out=wwq, in_=ps_ww)

            # ---- q^T projection: rstd * (Wq'^T x^T - wWq mu) + bWq ----
            qT = cst.tile([128, 2, NX], F32R)
            for dt in range(2):
                ps_q = psA.tile([128, NX], F32, tag="A")
                for t in range(8):
                    nc.tensor.matmul(
                        ps_q,
                        wq_t[:, t, dt * 128:(dt + 1) * 128],
                        xT_t[:, t],
                        start=(t == 0), stop=False,
                    )
                nc.tensor.matmul(
                    ps_q, wwq[:, dt * 128:(dt + 1) * 128], negmu, start=False, stop=True
                )
                nc.vector.tensor_mul(qT[:, dt], ps_q, rstd_bc)
                nc.vector.tensor_scalar_add(qT[:, dt], qT[:, dt], bwq[:, dt:dt + 1])

            # ---- natural-layout q (the appended-kv part) with ones column ----
            kvq = cst.tile([128, NKQ, HPC, HD + 1], F16)
            nc.gpsimd.memset(kvq[:, :, :, HD:HD + 1], 1.0)

            def build_kvq():
                for h in range(HPC):
                    pb = (h % 2) * 64
                    ps_t = psB.tile([128, NKQ * HD], F32, tag="B", name=f"tp{h}")
                    for tq in range(NKQ):
                        nc.tensor.transpose(
                            ps_t[:, tq * HD:(tq + 1) * HD],
                            qT[pb:pb + 64, h // 2, tq * 128:(tq + 1) * 128].bitcast(F32),
                            ident[pb:pb + 64, pb:pb + 64],
                        )
                    nc.vector.tensor_copy(
                        out=kvq[:, :, h, 0:HD],
                        in_=ps_t.rearrange("p (t d) -> p t d", d=HD),
                    )

            # ---- main attention: two passes, one head-pair each, so pass 0's
            # ---- normalize + dt=0 projection overlap pass 1's attention ----
            ps_aT = [psB.tile([HD + 1, NX], F32, tag="B", name=f"aT{i}") for i in range(HPC)]
            aTs = cst.tile([128, 2, NX], F32R)
            o0 = cst.tile([128, 4, 2, 512], F32)

            def scores_tile(kt, hp):
                ps_s = psA.tile([128, 2 * NX], F32, tag="A", name=f"s{hp}")
                for hh in range(2):
                    h = 2 * hp + hh
                    pb = (h % 2) * 64
                    if kt < NKQ:
                        lhsT = qT[pb:pb + 64, h // 2, kt * 128:(kt + 1) * 128]
                    else:
                        c0 = (kt - NKQ) * 128
                        lhsT = cT_t[pb:pb + 64, h // 2, c0:c0 + 128]
                    nc.tensor.matmul(
                        ps_s[:, hh * NX:(hh + 1) * NX],
                        lhsT,
                        qT[pb:pb + 64, h // 2, :],
                        start=True, stop=True,
                    )
                pt = ptp.tile([128, 2 * NX], F16, name=f"pt{hp}")
                nc.scalar.activation(out=pt, in_=ps_s, func=Exp, scale=SCALE)
                return pt

            def v_tile(kt, hp, pt):
                for hh in range(2):
                    h = 2 * hp + hh
                    if kt < NKQ:
                        lhsTv = kvq[:, kt, h, :]
                    else:
                        lhsTv = kvN[:, kt - NKQ, h, :]
                    nc.tensor.matmul(
                        ps_aT[h],
                        lhsTv,
                        pt[:, hh * NX:(hh + 1) * NX],
                        start=(kt == 0), stop=(kt == NKT - 1),
                    )

            def attn_tile(kt, pairs, tag):
                # one masked key tile for the given head pairs
                mf = mskp.tile([128, NX], F16, name=f"mf{tag}")
                nc.gpsimd.tensor_copy(out=mf, in_=mk_t[:, kt - NKQ])
                for hp in pairs:
                    pt = scores_tile(kt, hp)
                    ptv = pt.rearrange("p (a b) -> p a b", a=2)
                    nc.vector.tensor_mul(ptv, ptv, mf.rearrange("p (a b) -> p a b", a=1).broadcast_to((128, 2, NX)))
                    v_tile(kt, hp, pt)

            def normalize_pair(hp):
                # per-head softmax division at aT evacuation
                for hh in range(2):
                    h = 2 * hp + hh
                    pb = (h % 2) * 64
                    rc = rbp.tile([1, NX], F32, tag="rc", name=f"rc{h}")
                    nc.vector.reciprocal(out=rc, in_=ps_aT[h][HD:HD + 1, :])
                    ps_rc = psA.tile([64, NX], F32, tag="A", name=f"rcb{h}")
                    nc.tensor.matmul(ps_rc, ones_row[:, 0:64], rc, start=True, stop=True)
                    rb = rbp.tile([64, NX], F32, tag="rb", name=f"rb{h}")
                    nc.vector.tensor_copy(out=rb, in_=ps_rc)
                    nc.vector.tensor_mul(aTs[pb:pb + 64, h // 2, :], ps_aT[h][0:HD, :], rb)

            SPLIT = 30
            # q-region: scores+exp first (they only need qT), kvq transposes
            # overlap with the exps, then the q-region V matmuls
            q_pts = [(kt, hp, scores_tile(kt, hp)) for kt in range(NKQ) for hp in range(2)]
            build_kvq()
            for kt, hp, pt in q_pts:
                v_tile(kt, hp, pt)
            # phase 1: all four heads while inputs stream in
            for kt in range(NKQ, SPLIT):
                attn_tile(kt, (0, 1), "a")
            # phase 2: finish pair 0
            for kt in range(SPLIT, NKT):
                attn_tile(kt, (0,), "b")
            normalize_pair(0)
            # dt=0 half of the output projection overlaps phase 3
            for mt in range(4):
                for nt in range(2):
                    ps_o = psB.tile([128, 512], F32, tag="B", name="o0ps")
                    nc.tensor.matmul(
                        ps_o,
                        aTs[:, 0, mt * 128:(mt + 1) * 128],
                        wo_t[:, 0, nt * 512:(nt + 1) * 512],
                        start=True, stop=True,
                    )
                    nc.vector.tensor_copy(out=o0[:, mt, nt], in_=ps_o)
            # phase 3: finish pair 1
            for kt in range(SPLIT, NKT):
                attn_tile(kt, (1,), "c")
            normalize_pair(1)
            # tail: dt=1 matmul, add the dt=0 partial at evacuation, DMA out
            for mt in range(4):
                for nt in range(2):
                    ps_o = psB.tile([128, 512], F32, tag="B", name="o1ps")
                    nc.tensor.matmul(
                        ps_o,
                        aTs[:, 1, mt * 128:(mt + 1) * 128],
                        wo_t[:, 1, nt * 512:(nt + 1) * 512],
                        start=True, stop=True,
                    )
                    ob = osbp.tile([128, 512], F32)
                    nc.vector.tensor_add(ob, ps_o, o0[:, mt, nt])
                    nc.sync.dma_start(
                        out=o_out[mt * 128:(mt + 1) * 128, nt * 512:(nt + 1) * 512],
                        in_=ob,
                    )

    nc.compile()
    return nc


def _get_prog():
    global _PROG
    if _PROG is None:
        _PROG = _build_program()
    return _PROG


def kernel(x, c, attn_mask, ln_w, ln_b, Wq, Wo):
    global LAST_RESULT
    from concourse.bass_utils import run_bass_kernel_spmd

    x = np.asarray(x, dtype=np.float32)
    c = np.asarray(c, dtype=np.float32)
    mask = np.asarray(attn_mask)
    ln_w = np.asarray(ln_w, dtype=np.float32)
    ln_b = np.asarray(ln_b, dtype=np.float32)
    Wq = np.asarray(Wq, dtype=np.float32)
    Wo = np.asarray(Wo, dtype=np.float32)

    nc = _get_prog()

    in_maps = []
    for b in range(B):
        xTb = np.ascontiguousarray(x[b].T)
        cTb = np.ascontiguousarray(c[b].T)
        mkb = np.ascontiguousarray(mask[b].T).astype(np.uint8)
        for g in range(H // HPC):
            sl = slice(g * DL, (g + 1) * DL)
            cNp = np.ones((128, NKC, HPC, HD + 1), dtype=np.float16)
            cNp[:, :, :, :HD] = (
                c[b][:, sl]
                .reshape(NKC, 128, HPC, HD)
                .transpose(1, 0, 2, 3)
                .astype(np.float16)
            )
            in_maps.append({
                "xT": xTb,
                "cT": np.ascontiguousarray(cTb[sl]),
                "cN": cNp.reshape(128, NKC * HPC * (HD + 1)),
                "mk": mkb,
                "wq": np.ascontiguousarray(Wq[:, sl]),
                "wo": np.ascontiguousarray(Wo[sl, :]),
                "lnw": ln_w,
                "lnb": ln_b,
            })

    res = run_bass_kernel_spmd(nc, in_maps, core_ids=list(range(8)))
    LAST_RESULT = res

    o = np.zeros((B, NX, D), dtype=np.float32)
    for b in range(B):
        for g in range(H // HPC):
            o[b] += res.results[b * (H // HPC) + g]["o"]

    kv0 = np.ascontiguousarray(
        c.reshape(B, NCX, H, HD).transpose(0, 2, 1, 3)
    )
    return o, kv0


# revision 28
# speedup vs baseline: 1.0384x; 1.0384x over previous
"""CrossAttentionNoProj Trainium2 kernel.

Sharding (8 cores): 2-way data-parallel over batch x 4-way head-parallel
(4 heads / 256 inner dims per core). to_q column-parallel, to_out
row-parallel; per-core partial outputs are summed on gather.

Device algorithm per core (b, head-group g):
  - LayerNorm folded into the q-projection: stats (mean / E[x^2]) via
    ones-matmuls over x^T, q^T = rstd * (Wq'^T @ x^T - wWq outer mu) + bWq
    where Wq' = diag(ln_w) Wq, all computed in q-transposed layout so
    every matmul contracts along partitions.
  - Scores computed transposed (keys on partitions, queries on free axis):
    simT[k, m] = kvT . qT, one 128-row key tile per matmul, two heads
    sharing one 2-bank PSUM tile.
  - exp with the softmax scale folded into the ACT affine; mask applied
    multiplicatively after exp (equivalent since masked exp terms are
    exactly zeroed); kv-append-q handled by using on-chip q as the first
    4 key tiles (mask-free).
  - V-matmul in natural layout with a ones-column appended to kv so the
    softmax denominator falls out as row 64 of the accumulator.
  - Per-head division at aT evacuation, then the row-parallel output
    projection o_part = aT^T @ Wo[g-slice].
"""

import os
import numpy as np

# NTFF trace hooks are unavailable in this container; a stray BASS_TRACE
# would crash the run path, so pin it off unless the caller insists.
os.environ.setdefault("BASS_NEVER_TRACE", "1")

B, NX, NCX, D, H = 2, 512, 4096, 1024, 16
HD = 64
HPC = 4            # heads per core
DL = HPC * HD      # local inner dim = 256
SCALE = HD ** -0.5
LN_EPS = 1e-5
NKQ = NX // 128    # 4 key tiles from appended q
NKC = NCX // 128   # 32 key tiles from context
NKT = NKQ + NKC

_PROG = None
LAST_RESULT = None


def _build_program():
    from concourse import bacc
    import concourse.mybir as mybir
    from concourse.tile import TileContext
    from concourse.masks import make_identity

    F32 = mybir.dt.float32
    F32R = mybir.dt.float32r
    F16 = mybir.dt.float16
    U8 = mybir.dt.uint8

    def r(ap):
        return ap.bitcast(F32R)
    Exp = mybir.ActivationFunctionType.Exp
    Log = mybir.ActivationFunctionType.Ln

    nc = bacc.Bacc(None, target_bir_lowering=False, debug=False)

    xT_in = nc.dram_tensor("xT", (D, NX), F32R, kind="ExternalInput")
    cT_in = nc.dram_tensor("cT", (DL, NCX), F32R, kind="ExternalInput")
    cN_in = nc.dram_tensor("cN", (128, NKC * HPC * (HD + 1)), F16, kind="ExternalInput")
    mk_in = nc.dram_tensor("mk", (NCX, NX), U8, kind="ExternalInput")
    wq_in = nc.dram_tensor("wq", (D, DL), F32R, kind="ExternalInput")
    wo_in = nc.dram_tensor("wo", (DL, D), F32R, kind="ExternalInput")
    lnw_in = nc.dram_tensor("lnw", (D,), F32, kind="ExternalInput")
    lnb_in = nc.dram_tensor("lnb", (D,), F32R, kind="ExternalInput")
    o_out = nc.dram_tensor("o", (NX, D), F32, kind="ExternalOutput")

    with TileContext(nc) as tc:
        with (
            tc.tile_pool(name="cst", bufs=1) as cst,
            tc.tile_pool(name="xsq", bufs=2) as xsqp,
            tc.tile_pool(name="mskf", bufs=3) as mskp,
            tc.tile_pool(name="pt", bufs=6) as ptp,
            tc.tile_pool(name="rb", bufs=2) as rbp,
            tc.tile_pool(name="osb", bufs=4) as osbp,
            tc.tile_pool(name="psA", bufs=2, space="PSUM") as psA,
            tc.tile_pool(name="psB", bufs=4, space="PSUM") as psB,
        ):
            ident = cst.tile([128, 128], F32)
            make_identity(nc, ident)
            ones_col = cst.tile([128, 1], F32R)
            nc.vector.memset(ones_col.bitcast(mybir.dt.uint32), 0x3F800000)
            ones_row = cst.tile([1, 128], F32)
            nc.vector.memset(ones_row, 1.0)
            # dummy Ln+Exp so the ACT table set loads during the input DMAs
            warm = cst.tile([1, 2], F32)
            nc.vector.memset(warm, 1.0)
            nc.scalar.activation(out=warm[:, 0:1], in_=warm[:, 0:1], func=Log)
            nc.scalar.activation(out=warm[:, 1:2], in_=warm[:, 1:2], func=Exp)

            lnw_t = cst.tile([128, 8], F32)
            nc.sync.dma_start(out=lnw_t, in_=lnw_in[:].rearrange("(t p) -> p t", p=128))
            lnb_t = cst.tile([128, 8], F32R)
            nc.sync.dma_start(out=lnb_t, in_=lnb_in[:].rearrange("(t p) -> p t", p=128))
            xT_t = cst.tile([128, 8, NX], F32R)
            xT_re = xT_in[:].rearrange("(t p) n -> p t n", p=128)
            for t in range(0, 8, 2):
                nc.sync.dma_start(out=xT_t[:, t:t + 2], in_=xT_re[:, t:t + 2])
            wq_t = cst.tile([128, 8, DL], F32R)
            wq_re = wq_in[:].rearrange("(t p) n -> p t n", p=128)
            for t in range(0, 8, 4):
                nc.sync.dma_start(out=wq_t[:, t:t + 4], in_=wq_re[:, t:t + 4])
            cT_t = cst.tile([128, 2, NCX], F32R)
            cT_re = cT_in[:].rearrange("(t p) n -> p t n", p=128)
            mk_t = cst.tile([128, NKC, NX], U8)
            mk_re = mk_in[:].rearrange("(t p) n -> p t n", p=128)
            kvN = cst.tile([128, NKC, HPC, HD + 1], F16)
            # first score chunk + first mask chunk, then the whole kv (one
            # flat DMA — ones column pre-padded on host), then the rest
            nc.sync.dma_start(out=cT_t[:, :, 0:512], in_=cT_re[:, :, 0:512])
            nc.sync.dma_start(out=mk_t[:, 0:16], in_=mk_re[:, 0:16])
            nc.sync.dma_start(
                out=kvN.rearrange("p t h c -> p (t h c)"), in_=cN_in[:]
            )
            for ch in range(1, 8):
                nc.sync.dma_start(
                    out=cT_t[:, :, ch * 512:(ch + 1) * 512],
                    in_=cT_re[:, :, ch * 512:(ch + 1) * 512],
                )
            nc.sync.dma_start(out=mk_t[:, 16:32], in_=mk_re[:, 16:32])
            # wo is only needed for the tail projection — load it last
            wo_t = cst.tile([128, 2, D], F32R)
            wo_re = wo_in[:].rearrange("(t p) n -> p t n", p=128)
            for t in range(2):
                nc.sync.dma_start(out=wo_t[:, t], in_=wo_re[:, t])

            # ---- LayerNorm stats from x^T: mean and E[x^2] per token ----
            ps_su = psA.tile([1, NX], F32, tag="A")
            ps_sq = psA.tile([1, NX], F32, tag="A")
            for t in range(8):
                xsq = xsqp.tile([128, NX], F32R)
                nc.vector.tensor_mul(xsq, xT_t[:, t], xT_t[:, t])
                nc.tensor.matmul(ps_su, ones_col, xT_t[:, t], start=(t == 0), stop=(t == 7))
                nc.tensor.matmul(ps_sq, ones_col, xsq, start=(t == 0), stop=(t == 7))
            mu = cst.tile([1, NX], F32)
            nc.scalar.mul(out=mu, in_=ps_su, mul=1.0 / D)
            ex2 = cst.tile([1, NX], F32)
            nc.scalar.mul(out=ex2, in_=ps_sq, mul=1.0 / D)
            var = cst.tile([1, NX], F32)
            nc.vector.tensor_mul(var, mu, mu)
            nc.vector.tensor_sub(var, ex2, var)
            eps_t = cst.tile([1, 1], F32)
            nc.vector.memset(eps_t, LN_EPS)
            lnv = cst.tile([1, NX], F32)
            nc.scalar.activation(out=lnv, in_=var, func=Log, bias=eps_t)
            rstd = cst.tile([1, NX], F32)
            nc.scalar.activation(out=rstd, in_=lnv, func=Exp, scale=-0.5)
            negmu = cst.tile([1, NX], F32)
            nc.vector.tensor_scalar_mul(negmu, mu, -1.0)

            # ---- bWq row = ln_b @ Wq (raw Wq), ln_w fold into wq2,
            # ---- wWq row = colsum(wq2); all independent of the stats chain ----
            wq2 = cst.tile([128, 8, DL], F32R)
            for t in range(8):
                nc.vector.tensor_scalar_mul(wq2[:, t], wq_t[:, t], lnw_t[:, t:t + 1])
            ps_bw = psA.tile([1, DL], F32, tag="A")
            for t in range(8):
                nc.tensor.matmul(
                    ps_bw,
                    lnb_t[:, t:t + 1].bitcast(F32),
                    wq_t[:, t].bitcast(F32),
                    start=(t == 0), stop=(t == 7),
                )
            bwq = cst.tile([1, DL], F32)
            nc.vector.tensor_copy(out=bwq, in_=ps_bw)
            ps_ww = psA.tile([1, DL], F32, tag="A")
            for t in range(8):
                nc.tensor.matmul(ps_ww, ones_col, wq2[:, t], start=(t == 0), stop=(t == 7))
            wwq = cst.tile([1, DL], F32)
            nc.vector.tensor_copy(out=wwq, in_=ps_ww)
            rsc_tok = cst.tile([1, NX], F32)
            nc.vector.reciprocal(out=rsc_tok, in_=rstd)

            ps_rb = psA.tile([128, NX], F32, tag="A")
            nc.tensor.matmul(ps_rb, ones_row, rstd, start=True, stop=True)
            rstd_bc = cst.tile([128, NX], F32R)
            nc.vector.tensor_copy(out=rstd_bc, in_=ps_rb)
            # ---- q^T projection: rstd * (Wq'^T x^T - wWq mu + bWq/rstd) ----
            qT = cst.tile([128, 2, NX], F32R)
            for dt in range(2):
                ps_q = psA.tile([128, NX], F32, tag="A")
                for t in range(8):
                    nc.tensor.matmul(
                        ps_q,
                        wq2[:, t, dt * 128:(dt + 1) * 128],
                        xT_t[:, t],
                        start=(t == 0), stop=False,
                    )
                nc.tensor.matmul(
                    ps_q, wwq[:, dt * 128:(dt + 1) * 128], negmu, start=False, stop=False
                )
                nc.tensor.matmul(
                    ps_q, bwq[:, dt * 128:(dt + 1) * 128], rsc_tok, start=False, stop=True
                )
                nc.vector.tensor_mul(qT[:, dt], ps_q, rstd_bc)

            # ---- natural-layout q (the appended-kv part) with ones column ----
            kvq = cst.tile([128, NKQ, HPC, HD + 1], F16)
            nc.gpsimd.memset(kvq[:, :, :, HD:HD + 1], 1.0)

            def build_kvq():
                for h in range(HPC):
                    pb = (h % 2) * 64
                    ps_t = psB.tile([128, NKQ * HD], F32, tag="B", name=f"tp{h}")
                    for tq in range(NKQ):
                        nc.tensor.transpose(
                            ps_t[:, tq * HD:(tq + 1) * HD],
                            qT[pb:pb + 64, h // 2, tq * 128:(tq + 1) * 128].bitcast(F32),
                            ident[pb:pb + 64, pb:pb + 64],
                        )
                    nc.vector.tensor_copy(
                        out=kvq[:, :, h, 0:HD],
                        in_=ps_t.rearrange("p (t d) -> p t d", d=HD),
                    )

            # ---- main attention: two passes, one head-pair each, so pass 0's
            # ---- normalize + dt=0 projection overlap pass 1's attention ----
            ps_aT = [psB.tile([HD + 1, NX], F32, tag="B", name=f"aT{i}") for i in range(HPC)]
            aTs = cst.tile([128, 2, NX], F32R)
            o0 = cst.tile([128, 4, 2, 512], F32)

            def scores_tile(kt, hp):
                ps_s = psA.tile([128, 2 * NX], F32, tag="A", name=f"s{hp}")
                for hh in range(2):
                    h = 2 * hp + hh
                    pb = (h % 2) * 64
                    if kt < NKQ:
                        lhsT = qT[pb:pb + 64, h // 2, kt * 128:(kt + 1) * 128]
                    else:
                        c0 = (kt - NKQ) * 128
                        lhsT = cT_t[pb:pb + 64, h // 2, c0:c0 + 128]
                    nc.tensor.matmul(
                        ps_s[:, hh * NX:(hh + 1) * NX],
                        lhsT,
                        qT[pb:pb + 64, h // 2, :],
                        start=True, stop=True,
                    )
                pt = ptp.tile([128, 2 * NX], F16, name=f"pt{hp}")
                nc.scalar.activation(out=pt, in_=ps_s, func=Exp, scale=SCALE)
                return pt

            def v_tile(kt, hp, pt):
                for hh in range(2):
                    h = 2 * hp + hh
                    if kt < NKQ:
                        lhsTv = kvq[:, kt, h, :]
                    else:
                        lhsTv = kvN[:, kt - NKQ, h, :]
                    nc.tensor.matmul(
                        ps_aT[h],
                        lhsTv,
                        pt[:, hh * NX:(hh + 1) * NX],
                        start=(kt == 0), stop=(kt == NKT - 1),
                    )

            def attn_tile(kt, pairs, tag):
                # one masked key tile for the given head pairs
                mf = mskp.tile([128, NX], F16, name=f"mf{tag}")
                nc.gpsimd.tensor_copy(out=mf, in_=mk_t[:, kt - NKQ])
                for hp in pairs:
                    pt = scores_tile(kt, hp)
                    ptv = pt.rearrange("p (a b) -> p a b", a=2)
                    nc.vector.tensor_mul(ptv, ptv, mf.rearrange("p (a b) -> p a b", a=1).broadcast_to((128, 2, NX)))
                    v_tile(kt, hp, pt)

            def normalize_pair(hp):
                # per-head softmax division at aT evacuation
                for hh in range(2):
                    h = 2 * hp + hh
                    pb = (h % 2) * 64
                    rc = rbp.tile([1, NX], F32, tag="rc", name=f"rc{h}")
                    nc.vector.reciprocal(out=rc, in_=ps_aT[h][HD:HD + 1, :])
                    ps_rc = psA.tile([64, NX], F32, tag="A", name=f"rcb{h}")
                    nc.tensor.matmul(ps_rc, ones_row[:, 0:64], rc, start=True, stop=True)
                    rb = rbp.tile([64, NX], F32, tag="rb", name=f"rb{h}")
                    nc.vector.tensor_copy(out=rb, in_=ps_rc)
                    nc.vector.tensor_mul(aTs[pb:pb + 64, h // 2, :], ps_aT[h][0:HD, :], rb)

            SPLIT = 30
            # q-region: scores+exp first (they only need qT), kvq transposes
            # overlap with the exps, then the q-region V matmuls
            q_pts = [(kt, hp, scores_tile(kt, hp)) for hp in range(2) for kt in range(NKQ)]
            build_kvq()
            for kt, hp, pt in q_pts:
                v_tile(kt, hp, pt)
            # phase 1: all four heads while inputs stream in
            for kt in range(NKQ, SPLIT):
                attn_tile(kt, (0, 1), "a")
            # phase 2: finish pair 0
            for kt in range(SPLIT, NKT):
                attn_tile(kt, (0,), "b")
            normalize_pair(0)
            # dt=0 half of the output projection overlaps phase 3
            for mt in range(4):
                for nt in range(2):
                    ps_o = psB.tile([128, 512], F32, tag="B", name="o0ps")
                    nc.tensor.matmul(
                        ps_o,
                        aTs[:, 0, mt * 128:(mt + 1) * 128],
                        wo_t[:, 0, nt * 512:(nt + 1) * 512],
                        start=True, stop=True,
                    )
                    nc.vector.tensor_copy(out=o0[:, mt, nt], in_=ps_o)
            # phase 3: finish pair 1
            for kt in range(SPLIT, NKT):
                attn_tile(kt, (1,), "c")
            normalize_pair(1)
            # tail: dt=1 matmul, add the dt=0 partial at evacuation, DMA out
            for mt in range(4):
                for nt in range(2):
                    ps_o = psB.tile([128, 512], F32, tag="B", name="o1ps")
                    nc.tensor.matmul(
                        ps_o,
                        aTs[:, 1, mt * 128:(mt + 1) * 128],
                        wo_t[:, 1, nt * 512:(nt + 1) * 512],
                        start=True, stop=True,
                    )
                    ob = osbp.tile([128, 512], F32)
                    nc.vector.tensor_add(ob, ps_o, o0[:, mt, nt])
                    nc.sync.dma_start(
                        out=o_out[mt * 128:(mt + 1) * 128, nt * 512:(nt + 1) * 512],
                        in_=ob,
                    )

    nc.compile()
    return nc


def _get_prog():
    global _PROG
    if _PROG is None:
        _PROG = _build_program()
    return _PROG


def kernel(x, c, attn_mask, ln_w, ln_b, Wq, Wo):
    global LAST_RESULT
    from concourse.bass_utils import run_bass_kernel_spmd

    x = np.asarray(x, dtype=np.float32)
    c = np.asarray(c, dtype=np.float32)
    mask = np.asarray(attn_mask)
    ln_w = np.asarray(ln_w, dtype=np.float32)
    ln_b = np.asarray(ln_b, dtype=np.float32)
    Wq = np.asarray(Wq, dtype=np.float32)
    Wo = np.asarray(Wo, dtype=np.float32)

    nc = _get_prog()

    in_maps = []
    for b in range(B):
        xTb = np.ascontiguousarray(x[b].T)
        cTb = np.ascontiguousarray(c[b].T)
        mkb = np.ascontiguousarray(mask[b].T).astype(np.uint8)
        for g in range(H // HPC):
            sl = slice(g * DL, (g + 1) * DL)
            cNp = np.ones((128, NKC, HPC, HD + 1), dtype=np.float16)
            cNp[:, :, :, :HD] = (
                c[b][:, sl]
                .reshape(NKC, 128, HPC, HD)
                .transpose(1, 0, 2, 3)
                .astype(np.float16)
            )
            in_maps.append({
                "xT": xTb,
                "cT": np.ascontiguousarray(cTb[sl]),
                "cN": cNp.reshape(128, NKC * HPC * (HD + 1)),
                "mk": mkb,
                "wq": np.ascontiguousarray(Wq[:, sl]),
                "wo": np.ascontiguousarray(Wo[sl, :]),
                "lnw": ln_w,
                "lnb": ln_b,
            })

    res = run_bass_kernel_spmd(nc, in_maps, core_ids=list(range(8)))
    LAST_RESULT = res

    o = np.zeros((B, NX, D), dtype=np.float32)
    for b in range(B):
        for g in range(H // HPC):
            o[b] += res.results[b * (H // HPC) + g]["o"]

    kv0 = np.ascontiguousarray(
        c.reshape(B, NCX, H, HD).transpose(0, 2, 1, 3)
    )
    return o, kv0


# revision 35
# speedup vs baseline: 1.0388x; 1.0004x over previous
"""CrossAttentionNoProj Trainium2 kernel.

Sharding (8 cores): 2-way data-parallel over batch x 4-way head-parallel
(4 heads / 256 inner dims per core). to_q column-parallel, to_out
row-parallel; per-core partial outputs are summed on gather.

Device algorithm per core (b, head-group g):
  - LayerNorm folded into the q-projection: stats (mean / E[x^2]) via
    ones-matmuls over x^T, q^T = rstd * (Wq'^T @ x^T - wWq outer mu) + bWq
    where Wq' = diag(ln_w) Wq, all computed in q-transposed layout so
    every matmul contracts along partitions.
  - Scores computed transposed (keys on partitions, queries on free axis):
    simT[k, m] = kvT . qT, one 128-row key tile per matmul, two heads
    sharing one 2-bank PSUM tile.
  - exp with the softmax scale folded into the ACT affine; mask applied
    multiplicatively after exp (equivalent since masked exp terms are
    exactly zeroed); kv-append-q handled by using on-chip q as the first
    4 key tiles (mask-free).
  - V-matmul in natural layout with a ones-column appended to kv so the
    softmax denominator falls out as row 64 of the accumulator.
  - Per-head division at aT evacuation, then the row-parallel output
    projection o_part = aT^T @ Wo[g-slice].
"""

import os
import numpy as np

# NTFF trace hooks are unavailable in this container; a stray BASS_TRACE
# would crash the run path, so pin it off unless the caller insists.
os.environ.setdefault("BASS_NEVER_TRACE", "1")

B, NX, NCX, D, H = 2, 512, 4096, 1024, 16
HD = 64
HPC = 4            # heads per core
DL = HPC * HD      # local inner dim = 256
SCALE = HD ** -0.5
LN_EPS = 1e-5
NKQ = NX // 128    # 4 key tiles from appended q
NKC = NCX // 128   # 32 key tiles from context
NKT = NKQ + NKC

_PROG = None
LAST_RESULT = None


def _build_program():
    from concourse import bacc
    import concourse.mybir as mybir
    from concourse.tile import TileContext
    from concourse.masks import make_identity

    F32 = mybir.dt.float32
    F32R = mybir.dt.float32r
    F16 = mybir.dt.float16
    U8 = mybir.dt.uint8

    def r(ap):
        return ap.bitcast(F32R)
    Exp = mybir.ActivationFunctionType.Exp
    Log = mybir.ActivationFunctionType.Ln

    nc = bacc.Bacc(None, target_bir_lowering=False, debug=False)

    xT_in = nc.dram_tensor("xT", (D, NX), F32R, kind="ExternalInput")
    cT_in = nc.dram_tensor("cT", (DL, NCX), F32R, kind="ExternalInput")
    cN_in = nc.dram_tensor("cN", (128, NKC * HPC * (HD + 1)), F16, kind="ExternalInput")
    mk_in = nc.dram_tensor("mk", (NCX, NX), U8, kind="ExternalInput")
    wq_in = nc.dram_tensor("wq", (D, DL), F32R, kind="ExternalInput")
    wo_in = nc.dram_tensor("wo", (DL, D), F32R, kind="ExternalInput")
    lnw_in = nc.dram_tensor("lnw", (D,), F32, kind="ExternalInput")
    lnb_in = nc.dram_tensor("lnb", (D,), F32R, kind="ExternalInput")
    o_out = nc.dram_tensor("o", (NX, D), F32, kind="ExternalOutput")

    with TileContext(nc) as tc:
        with (
            tc.tile_pool(name="cst", bufs=1) as cst,
            tc.tile_pool(name="xsq", bufs=2) as xsqp,
            tc.tile_pool(name="mskf", bufs=3) as mskp,
            tc.tile_pool(name="pt", bufs=6) as ptp,
            tc.tile_pool(name="rb", bufs=2) as rbp,
            tc.tile_pool(name="osb", bufs=3) as osbp,
            tc.tile_pool(name="psA", bufs=2, space="PSUM") as psA,
            tc.tile_pool(name="psB", bufs=4, space="PSUM") as psB,
        ):
            ident = cst.tile([128, 128], F32)
            make_identity(nc, ident)
            ones_col = cst.tile([128, 1], F32R)
            nc.vector.memset(ones_col.bitcast(mybir.dt.uint32), 0x3F800000)
            ones_row = cst.tile([1, 128], F32)
            nc.vector.memset(ones_row, 1.0)
            # dummy Ln+Exp so the ACT table set loads during the input DMAs
            warm = cst.tile([1, 2], F32)
            nc.vector.memset(warm, 1.0)
            nc.scalar.activation(out=warm[:, 0:1], in_=warm[:, 0:1], func=Log)
            nc.scalar.activation(out=warm[:, 1:2], in_=warm[:, 1:2], func=Exp)

            lnw_t = cst.tile([128, 8], F32)
            nc.sync.dma_start(out=lnw_t, in_=lnw_in[:].rearrange("(t p) -> p t", p=128))
            lnb_t = cst.tile([128, 8], F32R)
            nc.sync.dma_start(out=lnb_t, in_=lnb_in[:].rearrange("(t p) -> p t", p=128))
            xT_t = cst.tile([128, 8, NX], F32R)
            xT_re = xT_in[:].rearrange("(t p) n -> p t n", p=128)
            for t in range(0, 8, 2):
                nc.sync.dma_start(out=xT_t[:, t:t + 2], in_=xT_re[:, t:t + 2])
            wq_t = cst.tile([128, 8, DL], F32R)
            wq_re = wq_in[:].rearrange("(t p) n -> p t n", p=128)
            for t in range(0, 8, 4):
                nc.sync.dma_start(out=wq_t[:, t:t + 4], in_=wq_re[:, t:t + 4])
            cT_t = cst.tile([128, 2, NCX], F32R)
            cT_re = cT_in[:].rearrange("(t p) n -> p t n", p=128)
            mk_t = cst.tile([128, NKC, NX], U8)
            mk_re = mk_in[:].rearrange("(t p) n -> p t n", p=128)
            kvN = cst.tile([128, NKC, HPC, HD + 1], F16)
            # first score chunk + first mask chunk, then the whole kv (one
            # flat DMA — ones column pre-padded on host), then the rest
            nc.sync.dma_start(out=cT_t[:, :, 0:512], in_=cT_re[:, :, 0:512])
            nc.sync.dma_start(out=mk_t[:, 0:16], in_=mk_re[:, 0:16])
            nc.sync.dma_start(
                out=kvN.rearrange("p t h c -> p (t h c)"), in_=cN_in[:]
            )
            for ch in range(1, 8):
                nc.sync.dma_start(
                    out=cT_t[:, :, ch * 512:(ch + 1) * 512],
                    in_=cT_re[:, :, ch * 512:(ch + 1) * 512],
                )
            nc.sync.dma_start(out=mk_t[:, 16:32], in_=mk_re[:, 16:32])
            # wo is only needed for the tail projection — load it last
            wo_t = cst.tile([128, 2, D], F32R)
            wo_re = wo_in[:].rearrange("(t p) n -> p t n", p=128)
            for t in range(2):
                nc.sync.dma_start(out=wo_t[:, t], in_=wo_re[:, t])

            # ---- LayerNorm stats from x^T: mean and E[x^2] per token ----
            ps_su = psA.tile([1, NX], F32, tag="A")
            ps_sq = psA.tile([1, NX], F32, tag="A")
            for t in range(8):
                xsq = xsqp.tile([128, NX], F32R)
                nc.vector.tensor_mul(xsq, xT_t[:, t], xT_t[:, t])
                nc.tensor.matmul(ps_su, ones_col, xT_t[:, t], start=(t == 0), stop=(t == 7))
                nc.tensor.matmul(ps_sq, ones_col, xsq, start=(t == 0), stop=(t == 7))
            mu = cst.tile([1, NX], F32)
            nc.scalar.mul(out=mu, in_=ps_su, mul=1.0 / D)
            ex2 = cst.tile([1, NX], F32)
            nc.scalar.mul(out=ex2, in_=ps_sq, mul=1.0 / D)
            var = cst.tile([1, NX], F32)
            nc.vector.tensor_mul(var, mu, mu)
            nc.vector.tensor_sub(var, ex2, var)
            eps_t = cst.tile([1, 1], F32)
            nc.vector.memset(eps_t, LN_EPS)
            lnv = cst.tile([1, NX], F32)
            nc.scalar.activation(out=lnv, in_=var, func=Log, bias=eps_t)
            rstd = cst.tile([1, NX], F32)
            nc.scalar.activation(out=rstd, in_=lnv, func=Exp, scale=-0.5)
            negmu = cst.tile([1, NX], F32)
            nc.vector.tensor_scalar_mul(negmu, mu, -1.0)

            # ---- bWq row = ln_b @ Wq (raw Wq), ln_w fold into wq2,
            # ---- wWq row = colsum(wq2); all independent of the stats chain ----
            wq2 = cst.tile([128, 8, DL], F32R)
            for t in range(8):
                nc.vector.tensor_scalar_mul(wq2[:, t], wq_t[:, t], lnw_t[:, t:t + 1])
            ps_bw = psA.tile([1, DL], F32, tag="A")
            for t in range(8):
                nc.tensor.matmul(
                    ps_bw,
                    lnb_t[:, t:t + 1].bitcast(F32),
                    wq_t[:, t].bitcast(F32),
                    start=(t == 0), stop=(t == 7),
                )
            bwq = cst.tile([1, DL], F32)
            nc.vector.tensor_copy(out=bwq, in_=ps_bw)
            ps_ww = psA.tile([1, DL], F32, tag="A")
            for t in range(8):
                nc.tensor.matmul(ps_ww, ones_col, wq2[:, t], start=(t == 0), stop=(t == 7))
            wwq = cst.tile([1, DL], F32)
            nc.vector.tensor_copy(out=wwq, in_=ps_ww)
            rsc_tok = cst.tile([1, NX], F32)
            nc.vector.reciprocal(out=rsc_tok, in_=rstd)

            ps_rb = psA.tile([128, NX], F32, tag="A")
            nc.tensor.matmul(ps_rb, ones_row, rstd, start=True, stop=True)
            rstd_bc = cst.tile([128, NX], F32R)
            nc.vector.tensor_copy(out=rstd_bc, in_=ps_rb)
            # ---- q^T projection: rstd * (Wq'^T x^T - wWq mu + bWq/rstd) ----
            qT = cst.tile([128, 2, NX], F32R)
            for dt in range(2):
                ps_q = psA.tile([128, NX], F32, tag="A")
                for t in range(8):
                    nc.tensor.matmul(
                        ps_q,
                        wq2[:, t, dt * 128:(dt + 1) * 128],
                        xT_t[:, t],
                        start=(t == 0), stop=False,
                    )
                nc.tensor.matmul(
                    ps_q, wwq[:, dt * 128:(dt + 1) * 128], negmu, start=False, stop=False
                )
                nc.tensor.matmul(
                    ps_q, bwq[:, dt * 128:(dt + 1) * 128], rsc_tok, start=False, stop=True
                )
                nc.vector.tensor_mul(qT[:, dt], ps_q, rstd_bc)

            # ---- natural-layout q (the appended-kv part) with ones column ----
            kvq = cst.tile([128, NKQ, HPC, HD + 1], F16)
            nc.gpsimd.memset(kvq[:, :, :, HD:HD + 1], 1.0)

            def build_kvq():
                for h in range(HPC):
                    pb = (h % 2) * 64
                    ps_t = psB.tile([128, NKQ * HD], F32, tag="B", name=f"tp{h}")
                    for tq in range(NKQ):
                        nc.tensor.transpose(
                            ps_t[:, tq * HD:(tq + 1) * HD],
                            qT[pb:pb + 64, h // 2, tq * 128:(tq + 1) * 128].bitcast(F32),
                            ident[pb:pb + 64, pb:pb + 64],
                        )
                    nc.vector.tensor_copy(
                        out=kvq[:, :, h, 0:HD],
                        in_=ps_t.rearrange("p (t d) -> p t d", d=HD),
                    )

            # ---- main attention: two passes, one head-pair each, so pass 0's
            # ---- normalize + dt=0 projection overlap pass 1's attention ----
            ps_aT = [psB.tile([HD + 1, NX], F32, tag="B", name=f"aT{i}") for i in range(HPC)]
            aTs = cst.tile([128, 2, NX], F32R)
            o0 = cst.tile([128, 4, 2, 512], F32)

            def scores_tile(kt, hp):
                ps_s = psA.tile([128, 2 * NX], F32, tag="A", name=f"s{hp}")
                for hh in range(2):
                    h = 2 * hp + hh
                    pb = (h % 2) * 64
                    if kt < NKQ:
                        lhsT = qT[pb:pb + 64, h // 2, kt * 128:(kt + 1) * 128]
                    else:
                        c0 = (kt - NKQ) * 128
                        lhsT = cT_t[pb:pb + 64, h // 2, c0:c0 + 128]
                    nc.tensor.matmul(
                        ps_s[:, hh * NX:(hh + 1) * NX],
                        lhsT,
                        qT[pb:pb + 64, h // 2, :],
                        start=True, stop=True,
                    )
                pt = ptp.tile([128, 2 * NX], F16, name=f"pt{hp}")
                nc.scalar.activation(out=pt, in_=ps_s, func=Exp, scale=SCALE)
                return pt

            def v_tile(kt, hp, pt):
                for hh in range(2):
                    h = 2 * hp + hh
                    if kt < NKQ:
                        lhsTv = kvq[:, kt, h, :]
                    else:
                        lhsTv = kvN[:, kt - NKQ, h, :]
                    nc.tensor.matmul(
                        ps_aT[h],
                        lhsTv,
                        pt[:, hh * NX:(hh + 1) * NX],
                        start=(kt == 0), stop=(kt == NKT - 1),
                    )

            def attn_tile(kt, pairs, tag):
                # one masked key tile for the given head pairs
                mf = mskp.tile([128, NX], F16, name=f"mf{tag}")
                nc.gpsimd.tensor_copy(out=mf, in_=mk_t[:, kt - NKQ])
                for hp in pairs:
                    pt = scores_tile(kt, hp)
                    ptv = pt.rearrange("p (a b) -> p a b", a=2)
                    nc.vector.tensor_mul(ptv, ptv, mf.rearrange("p (a b) -> p a b", a=1).broadcast_to((128, 2, NX)))
                    v_tile(kt, hp, pt)

            def normalize_pair(hp):
                # per-head softmax division at aT evacuation
                for hh in range(2):
                    h = 2 * hp + hh
                    pb = (h % 2) * 64
                    rc = rbp.tile([1, NX], F32, tag="rc", name=f"rc{h}")
                    nc.vector.reciprocal(out=rc, in_=ps_aT[h][HD:HD + 1, :])
                    ps_rc = psA.tile([64, NX], F32, tag="A", name=f"rcb{h}")
                    nc.tensor.matmul(ps_rc, ones_row[:, 0:64], rc, start=True, stop=True)
                    rb = rbp.tile([64, NX], F32, tag="rb", name=f"rb{h}")
                    nc.vector.tensor_copy(out=rb, in_=ps_rc)
                    nc.vector.tensor_mul(aTs[pb:pb + 64, h // 2, :], ps_aT[h][0:HD, :], rb)

            SPLIT = 30
            # q-region: scores+exp first (they only need qT), kvq transposes
            # overlap with the exps, then the q-region V matmuls
            q_pts = [(kt, hp, scores_tile(kt, hp)) for hp in range(2) for kt in range(NKQ)]
            build_kvq()
            for kt, hp, pt in q_pts:
                v_tile(kt, hp, pt)
            # phase 1: all four heads while inputs stream in
            for kt in range(NKQ, SPLIT):
                attn_tile(kt, (0, 1), "a")
            # phase 2: finish pair 0
            for kt in range(SPLIT, NKT):
                attn_tile(kt, (0,), "b")
            for kt in range(SPLIT, SPLIT + 3):
                attn_tile(kt, (1,), "c")
            normalize_pair(0)
            # dt=0 half of the output projection overlaps phase 3
            for mt in range(4):
                for nt in range(2):
                    ps_o = psB.tile([128, 512], F32, tag="B", name="o0ps")
                    nc.tensor.matmul(
                        ps_o,
                        aTs[:, 0, mt * 128:(mt + 1) * 128],
                        wo_t[:, 0, nt * 512:(nt + 1) * 512],
                        start=True, stop=True,
                    )
                    nc.vector.tensor_copy(out=o0[:, mt, nt], in_=ps_o)
            # phase 3: finish pair 1
            for kt in range(SPLIT + 3, NKT):
                attn_tile(kt, (1,), "c")
            normalize_pair(1)
            # tail: dt=1 matmul, add the dt=0 partial at evacuation, DMA out
            for mt in range(4):
                ob = osbp.tile([128, 2, 512], F32)
                for nt in range(2):
                    ps_o = psB.tile([128, 512], F32, tag="B", name="o1ps")
                    nc.tensor.matmul(
                        ps_o,
                        aTs[:, 1, mt * 128:(mt + 1) * 128],
                        wo_t[:, 1, nt * 512:(nt + 1) * 512],
                        start=True, stop=True,
                    )
                    nc.vector.tensor_add(ob[:, nt], ps_o, o0[:, mt, nt])
                nc.sync.dma_start(
                    out=o_out[mt * 128:(mt + 1) * 128, :],
                    in_=ob.rearrange("p a b -> p (a b)"),
                )

    nc.compile()
    return nc


def _get_prog():
    global _PROG
    if _PROG is None:
        _PROG = _build_program()
    return _PROG


def kernel(x, c, attn_mask, ln_w, ln_b, Wq, Wo):
    global LAST_RESULT
    from concourse.bass_utils import run_bass_kernel_spmd

    x = np.asarray(x, dtype=np.float32)
    c = np.asarray(c, dtype=np.float32)
    mask = np.asarray(attn_mask)
    ln_w = np.asarray(ln_w, dtype=np.float32)
    ln_b = np.asarray(ln_b, dtype=np.float32)
    Wq = np.asarray(Wq, dtype=np.float32)
    Wo = np.asarray(Wo, dtype=np.float32)

    nc = _get_prog()

    in_maps = []
    for b in range(B):
        xTb = np.ascontiguousarray(x[b].T)
        cTb = np.ascontiguousarray(c[b].T)
        mkb = np.ascontiguousarray(mask[b].T).astype(np.uint8)
        for g in range(H // HPC):
            sl = slice(g * DL, (g + 1) * DL)
            cNp = np.ones((128, NKC, HPC, HD + 1), dtype=np.float16)
            cNp[:, :, :, :HD] = (
                c[b][:, sl]
                .reshape(NKC, 128, HPC, HD)
                .transpose(1, 0, 2, 3)
                .astype(np.float16)
            )
            in_maps.append({
                "xT": xTb,
                "cT": np.ascontiguousarray(cTb[sl]),
                "cN": cNp.reshape(128, NKC * HPC * (HD + 1)),
                "mk": mkb,
                "wq": np.ascontiguousarray(Wq[:, sl]),
                "wo": np.ascontiguousarray(Wo[sl, :]),
                "lnw": ln_w,
                "lnb": ln_b,
            })

    res = run_bass_kernel_spmd(nc, in_maps, core_ids=list(range(8)))
    LAST_RESULT = res

    o = np.zeros((B, NX, D), dtype=np.float32)
    for b in range(B):
        for g in range(H // HPC):
            o[b] += res.results[b * (H // HPC) + g]["o"]

    kv0 = np.ascontiguousarray(
        c.reshape(B, NCX, H, HD).transpose(0, 2, 1, 3)
    )
    return o, kv0


# revision 41
# speedup vs baseline: 1.0453x; 1.0063x over previous
"""CrossAttentionNoProj Trainium2 kernel.

Sharding (8 cores): 2-way data-parallel over batch x 4-way head-parallel
(4 heads / 256 inner dims per core). to_q column-parallel, to_out
row-parallel; per-core partial outputs are summed on gather.

Device algorithm per core (b, head-group g):
  - LayerNorm folded into the q-projection: stats (mean / E[x^2]) via
    ones-matmuls over x^T, q^T = rstd * (Wq'^T @ x^T - wWq outer mu) + bWq
    where Wq' = diag(ln_w) Wq, all computed in q-transposed layout so
    every matmul contracts along partitions.
  - Scores computed transposed (keys on partitions, queries on free axis):
    simT[k, m] = kvT . qT, one 128-row key tile per matmul, two heads
    sharing one 2-bank PSUM tile.
  - exp with the softmax scale folded into the ACT affine; mask applied
    multiplicatively after exp (equivalent since masked exp terms are
    exactly zeroed); kv-append-q handled by using on-chip q as the first
    4 key tiles (mask-free).
  - V-matmul in natural layout with a ones-column appended to kv so the
    softmax denominator falls out as row 64 of the accumulator.
  - Per-head division at aT evacuation, then the row-parallel output
    projection o_part = aT^T @ Wo[g-slice].
"""

import os
import numpy as np

# NTFF trace hooks are unavailable in this container; a stray BASS_TRACE
# would crash the run path, so pin it off unless the caller insists.
os.environ.setdefault("BASS_NEVER_TRACE", "1")

B, NX, NCX, D, H = 2, 512, 4096, 1024, 16
HD = 64
HPC = 4            # heads per core
DL = HPC * HD      # local inner dim = 256
SCALE = HD ** -0.5
LN_EPS = 1e-5
NKQ = NX // 128    # 4 key tiles from appended q
NKC = NCX // 128   # 32 key tiles from context
NKT = NKQ + NKC

_PROG = None
LAST_RESULT = None


def _build_program():
    from concourse import bacc
    import concourse.mybir as mybir
    from concourse.tile import TileContext
    from concourse.masks import make_identity

    F32 = mybir.dt.float32
    F32R = mybir.dt.float32r
    F16 = mybir.dt.float16
    U8 = mybir.dt.uint8

    def r(ap):
        return ap.bitcast(F32R)
    Exp = mybir.ActivationFunctionType.Exp
    Log = mybir.ActivationFunctionType.Ln

    nc = bacc.Bacc(None, target_bir_lowering=False, debug=False)

    xT_in = nc.dram_tensor("xT", (D, NX), F32R, kind="ExternalInput")
    cT_in = nc.dram_tensor("cT", (DL, NCX), F32R, kind="ExternalInput")
    cN_in = nc.dram_tensor("cN", (128, NKC * HPC * (HD + 1)), F16, kind="ExternalInput")
    mk_in = nc.dram_tensor("mk", (NCX, NX), U8, kind="ExternalInput")
    wq_in = nc.dram_tensor("wq", (D, DL), F32R, kind="ExternalInput")
    wo_in = nc.dram_tensor("wo", (DL, D), F32R, kind="ExternalInput")
    lnw_in = nc.dram_tensor("lnw", (D,), F32, kind="ExternalInput")
    lnb_in = nc.dram_tensor("lnb", (D,), F32R, kind="ExternalInput")
    o_out = nc.dram_tensor("o", (NX, D), F32, kind="ExternalOutput")

    with TileContext(nc) as tc:
        with (
            tc.tile_pool(name="cst", bufs=1) as cst,
            tc.tile_pool(name="xsq", bufs=2) as xsqp,
            tc.tile_pool(name="mskf", bufs=3) as mskp,
            tc.tile_pool(name="pt", bufs=6) as ptp,
            tc.tile_pool(name="rb", bufs=2) as rbp,
            tc.tile_pool(name="osb", bufs=3) as osbp,
            tc.tile_pool(name="psA", bufs=2, space="PSUM") as psA,
            tc.tile_pool(name="psB", bufs=4, space="PSUM") as psB,
        ):
            ident = cst.tile([128, 128], F32)
            make_identity(nc, ident)
            ones_col = cst.tile([128, 1], F32R)
            nc.vector.memset(ones_col.bitcast(mybir.dt.uint32), 0x3F800000)
            ones_row = cst.tile([1, 128], F32)
            nc.vector.memset(ones_row, 1.0)
            # dummy Ln+Exp so the ACT table set loads during the input DMAs
            warm = cst.tile([1, 2], F32)
            nc.vector.memset(warm, 1.0)
            nc.scalar.activation(out=warm[:, 0:1], in_=warm[:, 0:1], func=Log)
            nc.scalar.activation(out=warm[:, 1:2], in_=warm[:, 1:2], func=Exp)

            lnw_t = cst.tile([128, 8], F32)
            nc.sync.dma_start(out=lnw_t, in_=lnw_in[:].rearrange("(t p) -> p t", p=128))
            lnb_t = cst.tile([128, 8], F32R)
            nc.sync.dma_start(out=lnb_t, in_=lnb_in[:].rearrange("(t p) -> p t", p=128))
            xT_t = cst.tile([128, 8, NX], F32R)
            xT_re = xT_in[:].rearrange("(t p) n -> p t n", p=128)
            for t in range(8):
                nc.sync.dma_start(out=xT_t[:, t], in_=xT_re[:, t])
            wq_t = cst.tile([128, 8, DL], F32R)
            wq_re = wq_in[:].rearrange("(t p) n -> p t n", p=128)
            for t in range(0, 8, 2):
                nc.sync.dma_start(out=wq_t[:, t:t + 2], in_=wq_re[:, t:t + 2])
            cT_t = cst.tile([128, 2, NCX], F32R)
            cT_re = cT_in[:].rearrange("(t p) n -> p t n", p=128)
            mk_t = cst.tile([128, NKC, NX], U8)
            mk_re = mk_in[:].rearrange("(t p) n -> p t n", p=128)
            kvN = cst.tile([128, NKC, HPC, HD + 1], F16)
            # first score chunk + first mask chunk, then the whole kv (one
            # flat DMA — ones column pre-padded on host), then the rest
            nc.sync.dma_start(out=cT_t[:, :, 0:256], in_=cT_re[:, :, 0:256])
            nc.sync.dma_start(out=cT_t[:, :, 256:512], in_=cT_re[:, :, 256:512])
            nc.sync.dma_start(out=mk_t[:, 0:16], in_=mk_re[:, 0:16])
            nc.sync.dma_start(
                out=kvN.rearrange("p t h c -> p (t h c)"), in_=cN_in[:]
            )
            for ch in range(1, 8):
                nc.sync.dma_start(
                    out=cT_t[:, :, ch * 512:(ch + 1) * 512],
                    in_=cT_re[:, :, ch * 512:(ch + 1) * 512],
                )
            nc.sync.dma_start(out=mk_t[:, 16:32], in_=mk_re[:, 16:32])
            # wo is only needed for the tail projection — load it last
            wo_t = cst.tile([128, 2, D], F32R)
            wo_re = wo_in[:].rearrange("(t p) n -> p t n", p=128)
            for t in range(2):
                nc.sync.dma_start(out=wo_t[:, t], in_=wo_re[:, t])

            # ---- LayerNorm stats from x^T: mean and E[x^2] per token ----
            ps_su = psA.tile([1, NX], F32, tag="A")
            ps_sq = psA.tile([1, NX], F32, tag="A")
            for t in range(8):
                xsq = xsqp.tile([128, NX], F32R)
                nc.vector.tensor_mul(xsq, xT_t[:, t], xT_t[:, t])
                nc.tensor.matmul(ps_su, ones_col, xT_t[:, t], start=(t == 0), stop=(t == 7))
                nc.tensor.matmul(ps_sq, ones_col, xsq, start=(t == 0), stop=(t == 7))
            mu = cst.tile([1, NX], F32)
            nc.scalar.mul(out=mu, in_=ps_su, mul=1.0 / D)
            ex2 = cst.tile([1, NX], F32)
            nc.scalar.mul(out=ex2, in_=ps_sq, mul=1.0 / D)
            var = cst.tile([1, NX], F32)
            nc.vector.tensor_mul(var, mu, mu)
            nc.vector.tensor_sub(var, ex2, var)
            eps_t = cst.tile([1, 1], F32)
            nc.vector.memset(eps_t, LN_EPS)
            lnv = cst.tile([1, NX], F32)
            nc.scalar.activation(out=lnv, in_=var, func=Log, bias=eps_t)
            rstd = cst.tile([1, NX], F32)
            nc.scalar.activation(out=rstd, in_=lnv, func=Exp, scale=-0.5)
            negmu = cst.tile([1, NX], F32)
            nc.vector.tensor_scalar_mul(negmu, mu, -1.0)

            # ---- bWq row = ln_b @ Wq (raw Wq), ln_w fold into wq2,
            # ---- wWq row = colsum(wq2); all independent of the stats chain ----
            wq2 = cst.tile([128, 8, DL], F32R)
            for t in range(8):
                nc.vector.tensor_scalar_mul(wq2[:, t], wq_t[:, t], lnw_t[:, t:t + 1])
            ps_bw = psA.tile([1, DL], F32, tag="A")
            for t in range(8):
                nc.tensor.matmul(
                    ps_bw,
                    lnb_t[:, t:t + 1].bitcast(F32),
                    wq_t[:, t].bitcast(F32),
                    start=(t == 0), stop=(t == 7),
                )
            bwq = cst.tile([1, DL], F32)
            nc.vector.tensor_copy(out=bwq, in_=ps_bw)
            ps_ww = psA.tile([1, DL], F32, tag="A")
            for t in range(8):
                nc.tensor.matmul(ps_ww, ones_col, wq2[:, t], start=(t == 0), stop=(t == 7))
            wwq = cst.tile([1, DL], F32)
            nc.vector.tensor_copy(out=wwq, in_=ps_ww)
            rsc_tok = cst.tile([1, NX], F32)
            nc.vector.reciprocal(out=rsc_tok, in_=rstd)

            ps_rb = psA.tile([128, NX], F32, tag="A")
            nc.tensor.matmul(ps_rb, ones_row, rstd, start=True, stop=True)
            rstd_bc = cst.tile([128, NX], F32R)
            nc.vector.tensor_copy(out=rstd_bc, in_=ps_rb)
            # ---- q^T projection: rstd * (Wq'^T x^T - wWq mu + bWq/rstd) ----
            qT = cst.tile([128, 2, NX], F32R)
            for dt in range(2):
                ps_q = psA.tile([128, NX], F32, tag="A")
                for t in range(8):
                    nc.tensor.matmul(
                        ps_q,
                        wq2[:, t, dt * 128:(dt + 1) * 128],
                        xT_t[:, t],
                        start=(t == 0), stop=False,
                    )
                nc.tensor.matmul(
                    ps_q, wwq[:, dt * 128:(dt + 1) * 128], negmu, start=False, stop=False
                )
                nc.tensor.matmul(
                    ps_q, bwq[:, dt * 128:(dt + 1) * 128], rsc_tok, start=False, stop=True
                )
                nc.vector.tensor_mul(qT[:, dt], ps_q, rstd_bc)

            # ---- natural-layout q (the appended-kv part) with ones column ----
            kvq = cst.tile([128, NKQ, HPC, HD + 1], F16)
            nc.gpsimd.memset(kvq[:, :, :, HD:HD + 1], 1.0)

            def build_kvq():
                for h in range(HPC):
                    pb = (h % 2) * 64
                    ps_t = psB.tile([128, NKQ * HD], F32, tag="B", name=f"tp{h}")
                    for tq in range(NKQ):
                        nc.tensor.transpose(
                            ps_t[:, tq * HD:(tq + 1) * HD],
                            qT[pb:pb + 64, h // 2, tq * 128:(tq + 1) * 128].bitcast(F32),
                            ident[pb:pb + 64, pb:pb + 64],
                        )
                    nc.vector.tensor_copy(
                        out=kvq[:, :, h, 0:HD],
                        in_=ps_t.rearrange("p (t d) -> p t d", d=HD),
                    )

            # ---- main attention: two passes, one head-pair each, so pass 0's
            # ---- normalize + dt=0 projection overlap pass 1's attention ----
            ps_aT = [psB.tile([HD + 1, NX], F32, tag="B", name=f"aT{i}") for i in range(HPC)]
            aTs = cst.tile([128, 2, NX], F32R)
            o0 = cst.tile([128, 4, 2, 512], F32)

            def scores_tile(kt, hp):
                ps_s = psA.tile([128, 2 * NX], F32, tag="A", name=f"s{hp}")
                for hh in range(2):
                    h = 2 * hp + hh
                    pb = (h % 2) * 64
                    if kt < NKQ:
                        lhsT = qT[pb:pb + 64, h // 2, kt * 128:(kt + 1) * 128]
                    else:
                        c0 = (kt - NKQ) * 128
                        lhsT = cT_t[pb:pb + 64, h // 2, c0:c0 + 128]
                    nc.tensor.matmul(
                        ps_s[:, hh * NX:(hh + 1) * NX],
                        lhsT,
                        qT[pb:pb + 64, h // 2, :],
                        start=True, stop=True,
                    )
                pt = ptp.tile([128, 2 * NX], F16, name=f"pt{hp}")
                nc.scalar.activation(out=pt, in_=ps_s, func=Exp, scale=SCALE)
                return pt

            def v_tile(kt, hp, pt):
                for hh in range(2):
                    h = 2 * hp + hh
                    if kt < NKQ:
                        lhsTv = kvq[:, kt, h, :]
                    else:
                        lhsTv = kvN[:, kt - NKQ, h, :]
                    nc.tensor.matmul(
                        ps_aT[h],
                        lhsTv,
                        pt[:, hh * NX:(hh + 1) * NX],
                        start=(kt == 0), stop=(kt == NKT - 1),
                    )

            def attn_tile(kt, pairs, tag):
                # one masked key tile for the given head pairs
                mf = mskp.tile([128, NX], F16, name=f"mf{tag}")
                nc.gpsimd.tensor_copy(out=mf, in_=mk_t[:, kt - NKQ])
                for hp in pairs:
                    pt = scores_tile(kt, hp)
                    ptv = pt.rearrange("p (a b) -> p a b", a=2)
                    nc.vector.tensor_mul(ptv, ptv, mf.rearrange("p (a b) -> p a b", a=1).broadcast_to((128, 2, NX)))
                    v_tile(kt, hp, pt)

            def normalize_pair(hp):
                # per-head softmax division at aT evacuation
                for hh in range(2):
                    h = 2 * hp + hh
                    pb = (h % 2) * 64
                    rc = rbp.tile([1, NX], F32, tag="rc", name=f"rc{h}")
                    nc.vector.reciprocal(out=rc, in_=ps_aT[h][HD:HD + 1, :])
                    ps_rc = psA.tile([64, NX], F32, tag="A", name=f"rcb{h}")
                    nc.tensor.matmul(ps_rc, ones_row[:, 0:64], rc, start=True, stop=True)
                    rb = rbp.tile([64, NX], F32, tag="rb", name=f"rb{h}")
                    nc.vector.tensor_copy(out=rb, in_=ps_rc)
                    nc.vector.tensor_mul(aTs[pb:pb + 64, h // 2, :], ps_aT[h][0:HD, :], rb)

            SPLIT = 30
            # q-region: scores+exp first (they only need qT), kvq transposes
            # overlap with the exps, then the q-region V matmuls
            q_pts = [(kt, hp, scores_tile(kt, hp)) for hp in range(2) for kt in range(NKQ)]
            build_kvq()
            for kt, hp, pt in q_pts:
                v_tile(kt, hp, pt)
            # phase 1: all four heads while inputs stream in
            for kt in range(NKQ, SPLIT):
                attn_tile(kt, (0, 1), "a")
            # phase 2: finish pair 0
            for kt in range(SPLIT, NKT):
                attn_tile(kt, (0,), "b")
            for kt in range(SPLIT, SPLIT + 3):
                attn_tile(kt, (1,), "c")
            normalize_pair(0)
            # dt=0 half of the output projection overlaps phase 3
            for mt in range(4):
                for nt in range(2):
                    ps_o = psB.tile([128, 512], F32, tag="B", name="o0ps")
                    nc.tensor.matmul(
                        ps_o,
                        aTs[:, 0, mt * 128:(mt + 1) * 128],
                        wo_t[:, 0, nt * 512:(nt + 1) * 512],
                        start=True, stop=True,
                    )
                    nc.vector.tensor_copy(out=o0[:, mt, nt], in_=ps_o)
            # phase 3: finish pair 1
            for kt in range(SPLIT + 3, NKT):
                attn_tile(kt, (1,), "c")
            normalize_pair(1)
            # tail: dt=1 matmul, add the dt=0 partial at evacuation, DMA out
            for mt in range(4):
                ob = osbp.tile([128, 2, 512], F32)
                for nt in range(2):
                    ps_o = psB.tile([128, 512], F32, tag="B", name="o1ps")
                    nc.tensor.matmul(
                        ps_o,
                        aTs[:, 1, mt * 128:(mt + 1) * 128],
                        wo_t[:, 1, nt * 512:(nt + 1) * 512],
                        start=True, stop=True,
                    )
                    nc.vector.tensor_add(ob[:, nt], ps_o, o0[:, mt, nt])
                nc.sync.dma_start(
                    out=o_out[mt * 128:(mt + 1) * 128, :],
                    in_=ob.rearrange("p a b -> p (a b)"),
                )

    nc.compile()
    return nc


def _get_prog():
    global _PROG
    if _PROG is None:
        _PROG = _build_program()
    return _PROG


def kernel(x, c, attn_mask, ln_w, ln_b, Wq, Wo):
    global LAST_RESULT
    from concourse.bass_utils import run_bass_kernel_spmd

    x = np.asarray(x, dtype=np.float32)
    c = np.asarray(c, dtype=np.float32)
    mask = np.asarray(attn_mask)
    ln_w = np.asarray(ln_w, dtype=np.float32)
    ln_b = np.asarray(ln_b, dtype=np.float32)
    Wq = np.asarray(Wq, dtype=np.float32)
    Wo = np.asarray(Wo, dtype=np.float32)

    nc = _get_prog()

    in_maps = []
    for b in range(B):
        xTb = np.ascontiguousarray(x[b].T)
        cTb = np.ascontiguousarray(c[b].T)
        mkb = np.ascontiguousarray(mask[b].T).astype(np.uint8)
        for g in range(H // HPC):
            sl = slice(g * DL, (g + 1) * DL)
            cNp = np.ones((128, NKC, HPC, HD + 1), dtype=np.float16)
            cNp[:, :, :, :HD] = (
                c[b][:, sl]
                .reshape(NKC, 128, HPC, HD)
                .transpose(1, 0, 2, 3)
                .astype(np.float16)
            )
            in_maps.append({
                "xT": xTb,
                "cT": np.ascontiguousarray(cTb[sl]),
                "cN": cNp.reshape(128, NKC * HPC * (HD + 1)),
                "mk": mkb,
                "wq": np.ascontiguousarray(Wq[:, sl]),
                "wo": np.ascontiguousarray(Wo[sl, :]),
                "lnw": ln_w,
                "lnb": ln_b,
            })

    res = run_bass_kernel_spmd(nc, in_maps, core_ids=list(range(8)))
    LAST_RESULT = res

    o = np.zeros((B, NX, D), dtype=np.float32)
    for b in range(B):
        for g in range(H // HPC):
            o[b] += res.results[b * (H // HPC) + g]["o"]

    kv0 = np.ascontiguousarray(
        c.reshape(B, NCX, H, HD).transpose(0, 2, 1, 3)
    )
    return o, kv0


# revision 42
# speedup vs baseline: 1.0522x; 1.0065x over previous
"""CrossAttentionNoProj Trainium2 kernel.

Sharding (8 cores): 2-way data-parallel over batch x 4-way head-parallel
(4 heads / 256 inner dims per core). to_q column-parallel, to_out
row-parallel; per-core partial outputs are summed on gather.

Device algorithm per core (b, head-group g):
  - LayerNorm folded into the q-projection: stats (mean / E[x^2]) via
    ones-matmuls over x^T, q^T = rstd * (Wq'^T @ x^T - wWq outer mu) + bWq
    where Wq' = diag(ln_w) Wq, all computed in q-transposed layout so
    every matmul contracts along partitions.
  - Scores computed transposed (keys on partitions, queries on free axis):
    simT[k, m] = kvT . qT, one 128-row key tile per matmul, two heads
    sharing one 2-bank PSUM tile.
  - exp with the softmax scale folded into the ACT affine; mask applied
    multiplicatively after exp (equivalent since masked exp terms are
    exactly zeroed); kv-append-q handled by using on-chip q as the first
    4 key tiles (mask-free).
  - V-matmul in natural layout with a ones-column appended to kv so the
    softmax denominator falls out as row 64 of the accumulator.
  - Per-head division at aT evacuation, then the row-parallel output
    projection o_part = aT^T @ Wo[g-slice].
"""

import os
import numpy as np

# NTFF trace hooks are unavailable in this container; a stray BASS_TRACE
# would crash the run path, so pin it off unless the caller insists.
os.environ.setdefault("BASS_NEVER_TRACE", "1")

B, NX, NCX, D, H = 2, 512, 4096, 1024, 16
HD = 64
HPC = 4            # heads per core
DL = HPC * HD      # local inner dim = 256
SCALE = HD ** -0.5
LN_EPS = 1e-5
NKQ = NX // 128    # 4 key tiles from appended q
NKC = NCX // 128   # 32 key tiles from context
NKT = NKQ + NKC

_PROG = None
LAST_RESULT = None


def _build_program():
    from concourse import bacc
    import concourse.mybir as mybir
    from concourse.tile import TileContext
    from concourse.masks import make_identity

    F32 = mybir.dt.float32
    F32R = mybir.dt.float32r
    F16 = mybir.dt.float16
    U8 = mybir.dt.uint8

    def r(ap):
        return ap.bitcast(F32R)
    Exp = mybir.ActivationFunctionType.Exp
    Log = mybir.ActivationFunctionType.Ln

    nc = bacc.Bacc(None, target_bir_lowering=False, debug=False)

    xT_in = nc.dram_tensor("xT", (D, NX), F32R, kind="ExternalInput")
    cT_in = nc.dram_tensor("cT", (DL, NCX), F32R, kind="ExternalInput")
    cN_in = nc.dram_tensor("cN", (128, NKC * HPC * (HD + 1)), F16, kind="ExternalInput")
    mk_in = nc.dram_tensor("mk", (NCX, NX), U8, kind="ExternalInput")
    wq_in = nc.dram_tensor("wq", (D, DL), F32R, kind="ExternalInput")
    wo_in = nc.dram_tensor("wo", (DL, D), F32R, kind="ExternalInput")
    lnw_in = nc.dram_tensor("lnw", (D,), F32, kind="ExternalInput")
    lnb_in = nc.dram_tensor("lnb", (D,), F32R, kind="ExternalInput")
    o_out = nc.dram_tensor("o", (NX, D), F32, kind="ExternalOutput")

    with TileContext(nc) as tc:
        with (
            tc.tile_pool(name="cst", bufs=1) as cst,
            tc.tile_pool(name="xsq", bufs=2) as xsqp,
            tc.tile_pool(name="mskf", bufs=3) as mskp,
            tc.tile_pool(name="pt", bufs=6) as ptp,
            tc.tile_pool(name="rb", bufs=2) as rbp,
            tc.tile_pool(name="osb", bufs=3) as osbp,
            tc.tile_pool(name="psA", bufs=2, space="PSUM") as psA,
            tc.tile_pool(name="psB", bufs=4, space="PSUM") as psB,
        ):
            ident = cst.tile([128, 128], F32)
            make_identity(nc, ident)
            ones_col = cst.tile([128, 1], F32R)
            nc.vector.memset(ones_col.bitcast(mybir.dt.uint32), 0x3F800000)
            ones_row = cst.tile([1, 128], F32)
            nc.vector.memset(ones_row, 1.0)
            ones_rowr = cst.tile([1, 64], F32R)
            nc.vector.memset(ones_rowr.bitcast(mybir.dt.uint32), 0x3F800000)
            # dummy Ln+Exp so the ACT table set loads during the input DMAs
            warm = cst.tile([1, 2], F32)
            nc.vector.memset(warm, 1.0)
            nc.scalar.activation(out=warm[:, 0:1], in_=warm[:, 0:1], func=Log)
            nc.scalar.activation(out=warm[:, 1:2], in_=warm[:, 1:2], func=Exp)

            lnw_t = cst.tile([128, 8], F32)
            nc.sync.dma_start(out=lnw_t, in_=lnw_in[:].rearrange("(t p) -> p t", p=128))
            lnb_t = cst.tile([128, 8], F32R)
            nc.sync.dma_start(out=lnb_t, in_=lnb_in[:].rearrange("(t p) -> p t", p=128))
            xT_t = cst.tile([128, 8, NX], F32R)
            xT_re = xT_in[:].rearrange("(t p) n -> p t n", p=128)
            for t in range(8):
                nc.sync.dma_start(out=xT_t[:, t], in_=xT_re[:, t])
            wq_t = cst.tile([128, 8, DL], F32R)
            wq_re = wq_in[:].rearrange("(t p) n -> p t n", p=128)
            for t in range(0, 8, 2):
                nc.sync.dma_start(out=wq_t[:, t:t + 2], in_=wq_re[:, t:t + 2])
            cT_t = cst.tile([128, 2, NCX], F32R)
            cT_re = cT_in[:].rearrange("(t p) n -> p t n", p=128)
            mk_t = cst.tile([128, NKC, NX], U8)
            mk_re = mk_in[:].rearrange("(t p) n -> p t n", p=128)
            kvN = cst.tile([128, NKC, HPC, HD + 1], F16)
            # first score chunk + first mask chunk, then the whole kv (one
            # flat DMA — ones column pre-padded on host), then the rest
            nc.sync.dma_start(out=cT_t[:, :, 0:256], in_=cT_re[:, :, 0:256])
            nc.sync.dma_start(out=cT_t[:, :, 256:512], in_=cT_re[:, :, 256:512])
            nc.sync.dma_start(out=mk_t[:, 0:16], in_=mk_re[:, 0:16])
            nc.sync.dma_start(
                out=kvN.rearrange("p t h c -> p (t h c)"), in_=cN_in[:]
            )
            for ch in range(1, 8):
                nc.sync.dma_start(
                    out=cT_t[:, :, ch * 512:(ch + 1) * 512],
                    in_=cT_re[:, :, ch * 512:(ch + 1) * 512],
                )
            nc.sync.dma_start(out=mk_t[:, 16:32], in_=mk_re[:, 16:32])
            # wo is only needed for the tail projection — load it last
            wo_t = cst.tile([128, 2, D], F32R)
            wo_re = wo_in[:].rearrange("(t p) n -> p t n", p=128)
            for t in range(2):
                nc.sync.dma_start(out=wo_t[:, t], in_=wo_re[:, t])

            # ---- LayerNorm stats from x^T: mean and E[x^2] per token ----
            ps_su = psA.tile([1, NX], F32, tag="A")
            ps_sq = psA.tile([1, NX], F32, tag="A")
            for t in range(8):
                xsq = xsqp.tile([128, NX], F32R)
                nc.vector.tensor_mul(xsq, xT_t[:, t], xT_t[:, t])
                nc.tensor.matmul(ps_su, ones_col, xT_t[:, t], start=(t == 0), stop=(t == 7))
                nc.tensor.matmul(ps_sq, ones_col, xsq, start=(t == 0), stop=(t == 7))
            mu = cst.tile([1, NX], F32)
            nc.scalar.mul(out=mu, in_=ps_su, mul=1.0 / D)
            ex2 = cst.tile([1, NX], F32)
            nc.scalar.mul(out=ex2, in_=ps_sq, mul=1.0 / D)
            var = cst.tile([1, NX], F32)
            nc.vector.tensor_mul(var, mu, mu)
            nc.vector.tensor_sub(var, ex2, var)
            eps_t = cst.tile([1, 1], F32)
            nc.vector.memset(eps_t, LN_EPS)
            lnv = cst.tile([1, NX], F32)
            nc.scalar.activation(out=lnv, in_=var, func=Log, bias=eps_t)
            rstd = cst.tile([1, NX], F32)
            nc.scalar.activation(out=rstd, in_=lnv, func=Exp, scale=-0.5)
            negmu = cst.tile([1, NX], F32)
            nc.vector.tensor_scalar_mul(negmu, mu, -1.0)

            # ---- bWq row = ln_b @ Wq (raw Wq), ln_w fold into wq2,
            # ---- wWq row = colsum(wq2); all independent of the stats chain ----
            wq2 = cst.tile([128, 8, DL], F32R)
            for t in range(8):
                nc.vector.tensor_scalar_mul(wq2[:, t], wq_t[:, t], lnw_t[:, t:t + 1])
            ps_bw = psA.tile([1, DL], F32, tag="A")
            for t in range(8):
                nc.tensor.matmul(
                    ps_bw,
                    lnb_t[:, t:t + 1].bitcast(F32),
                    wq_t[:, t].bitcast(F32),
                    start=(t == 0), stop=(t == 7),
                )
            bwq = cst.tile([1, DL], F32)
            nc.vector.tensor_copy(out=bwq, in_=ps_bw)
            ps_ww = psA.tile([1, DL], F32, tag="A")
            for t in range(8):
                nc.tensor.matmul(ps_ww, ones_col, wq2[:, t], start=(t == 0), stop=(t == 7))
            wwq = cst.tile([1, DL], F32)
            nc.vector.tensor_copy(out=wwq, in_=ps_ww)
            rsc_tok = cst.tile([1, NX], F32)
            nc.vector.reciprocal(out=rsc_tok, in_=rstd)

            ps_rb = psA.tile([128, NX], F32, tag="A")
            nc.tensor.matmul(ps_rb, ones_row, rstd, start=True, stop=True)
            rstd_bc = cst.tile([128, NX], F32R)
            nc.vector.tensor_copy(out=rstd_bc, in_=ps_rb)
            # ---- q^T projection: rstd * (Wq'^T x^T - wWq mu + bWq/rstd) ----
            qT = cst.tile([128, 2, NX], F32R)
            for dt in range(2):
                ps_q = psA.tile([128, NX], F32, tag="A")
                for t in range(8):
                    nc.tensor.matmul(
                        ps_q,
                        wq2[:, t, dt * 128:(dt + 1) * 128],
                        xT_t[:, t],
                        start=(t == 0), stop=False,
                    )
                nc.tensor.matmul(
                    ps_q, wwq[:, dt * 128:(dt + 1) * 128], negmu, start=False, stop=False
                )
                nc.tensor.matmul(
                    ps_q, bwq[:, dt * 128:(dt + 1) * 128], rsc_tok, start=False, stop=True
                )
                nc.vector.tensor_mul(qT[:, dt], ps_q, rstd_bc)

            # ---- natural-layout q (the appended-kv part) with ones column ----
            kvq = cst.tile([128, NKQ, HPC, HD + 1], F16)
            nc.gpsimd.memset(kvq[:, :, :, HD:HD + 1], 1.0)

            def build_kvq():
                for h in range(HPC):
                    pb = (h % 2) * 64
                    ps_t = psB.tile([128, NKQ * HD], F32, tag="B", name=f"tp{h}")
                    for tq in range(NKQ):
                        nc.tensor.transpose(
                            ps_t[:, tq * HD:(tq + 1) * HD],
                            qT[pb:pb + 64, h // 2, tq * 128:(tq + 1) * 128].bitcast(F32),
                            ident[pb:pb + 64, pb:pb + 64],
                        )
                    nc.vector.tensor_copy(
                        out=kvq[:, :, h, 0:HD],
                        in_=ps_t.rearrange("p (t d) -> p t d", d=HD),
                    )

            # ---- main attention: two passes, one head-pair each, so pass 0's
            # ---- normalize + dt=0 projection overlap pass 1's attention ----
            ps_aT = [psB.tile([HD + 1, NX], F32, tag="B", name=f"aT{i}") for i in range(HPC)]
            aTs = cst.tile([128, 2, NX], F32R)
            o0 = cst.tile([128, 4, 2, 512], F32)

            def scores_tile(kt, hp):
                ps_s = psA.tile([128, 2 * NX], F32, tag="A", name=f"s{hp}")
                for hh in range(2):
                    h = 2 * hp + hh
                    pb = (h % 2) * 64
                    if kt < NKQ:
                        lhsT = qT[pb:pb + 64, h // 2, kt * 128:(kt + 1) * 128]
                    else:
                        c0 = (kt - NKQ) * 128
                        lhsT = cT_t[pb:pb + 64, h // 2, c0:c0 + 128]
                    nc.tensor.matmul(
                        ps_s[:, hh * NX:(hh + 1) * NX],
                        lhsT,
                        qT[pb:pb + 64, h // 2, :],
                        start=True, stop=True,
                    )
                pt = ptp.tile([128, 2 * NX], F16, name=f"pt{hp}")
                nc.scalar.activation(out=pt, in_=ps_s, func=Exp, scale=SCALE)
                return pt

            def v_tile(kt, hp, pt):
                for hh in range(2):
                    h = 2 * hp + hh
                    if kt < NKQ:
                        lhsTv = kvq[:, kt, h, :]
                    else:
                        lhsTv = kvN[:, kt - NKQ, h, :]
                    nc.tensor.matmul(
                        ps_aT[h],
                        lhsTv,
                        pt[:, hh * NX:(hh + 1) * NX],
                        start=(kt == 0), stop=(kt == NKT - 1),
                    )

            def attn_tile(kt, pairs, tag):
                # one masked key tile for the given head pairs
                mf = mskp.tile([128, NX], F16, name=f"mf{tag}")
                nc.gpsimd.tensor_copy(out=mf, in_=mk_t[:, kt - NKQ])
                for hp in pairs:
                    pt = scores_tile(kt, hp)
                    ptv = pt.rearrange("p (a b) -> p a b", a=2)
                    nc.vector.tensor_mul(ptv, ptv, mf.rearrange("p (a b) -> p a b", a=1).broadcast_to((128, 2, NX)))
                    v_tile(kt, hp, pt)

            def normalize_pair(hp):
                # per-head softmax division at aT evacuation; emit both heads'
                # recips first so neither blocks the other behind the DVE FIFO
                rcs, pss = [], []
                with nc.allow_low_precision(reason="fp32r recip feeds fp32r bcast matmul"):
                    for hh in range(2):
                        h = 2 * hp + hh
                        rc = rbp.tile([1, NX], F32R, tag="rc", name=f"rc{h}")
                        nc.vector.reciprocal(out=rc, in_=ps_aT[h][HD:HD + 1, :])
                        rcs.append(rc)
                for hh in range(2):
                    h = 2 * hp + hh
                    ps_rc = psA.tile([64, NX], F32, tag="A", name=f"rcb{h}")
                    nc.tensor.matmul(ps_rc, ones_rowr, rcs[hh], start=True, stop=True)
                    pss.append(ps_rc)
                for hh in range(2):
                    h = 2 * hp + hh
                    pb = (h % 2) * 64
                    rb = rbp.tile([64, NX], F32, tag="rb", name=f"rb{h}")
                    nc.vector.tensor_copy(out=rb, in_=pss[hh])
                    nc.vector.tensor_mul(aTs[pb:pb + 64, h // 2, :], ps_aT[h][0:HD, :], rb)

            SPLIT = 30
            # q-region: scores+exp first (they only need qT), kvq transposes
            # overlap with the exps, then the q-region V matmuls
            q_pts = [(kt, hp, scores_tile(kt, hp)) for hp in range(2) for kt in range(NKQ)]
            build_kvq()
            for kt, hp, pt in q_pts:
                v_tile(kt, hp, pt)
            # phase 1: all four heads while inputs stream in
            for kt in range(NKQ, SPLIT):
                attn_tile(kt, (0, 1), "a")
            # phase 2: finish pair 0
            for kt in range(SPLIT, NKT):
                attn_tile(kt, (0,), "b")
            for kt in range(SPLIT, SPLIT + 3):
                attn_tile(kt, (1,), "c")
            normalize_pair(0)
            # dt=0 half of the output projection overlaps phase 3
            for mt in range(4):
                for nt in range(2):
                    ps_o = psB.tile([128, 512], F32, tag="B", name="o0ps")
                    nc.tensor.matmul(
                        ps_o,
                        aTs[:, 0, mt * 128:(mt + 1) * 128],
                        wo_t[:, 0, nt * 512:(nt + 1) * 512],
                        start=True, stop=True,
                    )
                    nc.vector.tensor_copy(out=o0[:, mt, nt], in_=ps_o)
            # phase 3: finish pair 1
            for kt in range(SPLIT + 3, NKT):
                attn_tile(kt, (1,), "c")
            normalize_pair(1)
            # tail: dt=1 matmul, add the dt=0 partial at evacuation, DMA out
            for mt in range(4):
                ob = osbp.tile([128, 2, 512], F32)
                for nt in range(2):
                    ps_o = psB.tile([128, 512], F32, tag="B", name="o1ps")
                    nc.tensor.matmul(
                        ps_o,
                        aTs[:, 1, mt * 128:(mt + 1) * 128],
                        wo_t[:, 1, nt * 512:(nt + 1) * 512],
                        start=True, stop=True,
                    )
                    nc.vector.tensor_add(ob[:, nt], ps_o, o0[:, mt, nt])
                nc.sync.dma_start(
                    out=o_out[mt * 128:(mt + 1) * 128, :],
                    in_=ob.rearrange("p a b -> p (a b)"),
                )

    nc.compile()
    return nc


def _get_prog():
    global _PROG
    if _PROG is None:
        _PROG = _build_program()
    return _PROG


def kernel(x, c, attn_mask, ln_w, ln_b, Wq, Wo):
    global LAST_RESULT
    from concourse.bass_utils import run_bass_kernel_spmd

    x = np.asarray(x, dtype=np.float32)
    c = np.asarray(c, dtype=np.float32)
    mask = np.asarray(attn_mask)
    ln_w = np.asarray(ln_w, dtype=np.float32)
    ln_b = np.asarray(ln_b, dtype=np.float32)
    Wq = np.asarray(Wq, dtype=np.float32)
    Wo = np.asarray(Wo, dtype=np.float32)

    nc = _get_prog()

    in_maps = []
    for b in range(B):
        xTb = np.ascontiguousarray(x[b].T)
        cTb = np.ascontiguousarray(c[b].T)
        mkb = np.ascontiguousarray(mask[b].T).astype(np.uint8)
        for g in range(H // HPC):
            sl = slice(g * DL, (g + 1) * DL)
            cNp = np.ones((128, NKC, HPC, HD + 1), dtype=np.float16)
            cNp[:, :, :, :HD] = (
                c[b][:, sl]
                .reshape(NKC, 128, HPC, HD)
                .transpose(1, 0, 2, 3)
                .astype(np.float16)
            )
            in_maps.append({
                "xT": xTb,
                "cT": np.ascontiguousarray(cTb[sl]),
                "cN": cNp.reshape(128, NKC * HPC * (HD + 1)),
                "mk": mkb,
                "wq": np.ascontiguousarray(Wq[:, sl]),
                "wo": np.ascontiguousarray(Wo[sl, :]),
                "lnw": ln_w,
                "lnb": ln_b,
            })

    res = run_bass_kernel_spmd(nc, in_maps, core_ids=list(range(8)))
    LAST_RESULT = res

    o = np.zeros((B, NX, D), dtype=np.float32)
    for b in range(B):
        for g in range(H // HPC):
            o[b] += res.results[b * (H // HPC) + g]["o"]

    kv0 = np.ascontiguousarray(
        c.reshape(B, NCX, H, HD).transpose(0, 2, 1, 3)
    )
    return o, kv0


# revision 43
# speedup vs baseline: 1.0581x; 1.0056x over previous
"""CrossAttentionNoProj Trainium2 kernel.

Sharding (8 cores): 2-way data-parallel over batch x 4-way head-parallel
(4 heads / 256 inner dims per core). to_q column-parallel, to_out
row-parallel; per-core partial outputs are summed on gather.

Device algorithm per core (b, head-group g):
  - LayerNorm folded into the q-projection: stats (mean / E[x^2]) via
    ones-matmuls over x^T, q^T = rstd * (Wq'^T @ x^T - wWq outer mu) + bWq
    where Wq' = diag(ln_w) Wq, all computed in q-transposed layout so
    every matmul contracts along partitions.
  - Scores computed transposed (keys on partitions, queries on free axis):
    simT[k, m] = kvT . qT, one 128-row key tile per matmul, two heads
    sharing one 2-bank PSUM tile.
  - exp with the softmax scale folded into the ACT affine; mask applied
    multiplicatively after exp (equivalent since masked exp terms are
    exactly zeroed); kv-append-q handled by using on-chip q as the first
    4 key tiles (mask-free).
  - V-matmul in natural layout with a ones-column appended to kv so the
    softmax denominator falls out as row 64 of the accumulator.
  - Per-head division at aT evacuation, then the row-parallel output
    projection o_part = aT^T @ Wo[g-slice].
"""

import os
import numpy as np

# NTFF trace hooks are unavailable in this container; a stray BASS_TRACE
# would crash the run path, so pin it off unless the caller insists.
os.environ.setdefault("BASS_NEVER_TRACE", "1")

B, NX, NCX, D, H = 2, 512, 4096, 1024, 16
HD = 64
HPC = 4            # heads per core
DL = HPC * HD      # local inner dim = 256
SCALE = HD ** -0.5
LN_EPS = 1e-5
NKQ = NX // 128    # 4 key tiles from appended q
NKC = NCX // 128   # 32 key tiles from context
NKT = NKQ + NKC

_PROG = None
LAST_RESULT = None


def _build_program():
    from concourse import bacc
    import concourse.mybir as mybir
    from concourse.tile import TileContext
    from concourse.masks import make_identity

    F32 = mybir.dt.float32
    F32R = mybir.dt.float32r
    F16 = mybir.dt.float16
    U8 = mybir.dt.uint8

    def r(ap):
        return ap.bitcast(F32R)
    Exp = mybir.ActivationFunctionType.Exp
    Log = mybir.ActivationFunctionType.Ln

    nc = bacc.Bacc(None, target_bir_lowering=False, debug=False)

    xT_in = nc.dram_tensor("xT", (D, NX), F32R, kind="ExternalInput")
    cT_in = nc.dram_tensor("cT", (DL, NCX), F32R, kind="ExternalInput")
    cN_in = nc.dram_tensor("cN", (128, NKC * HPC * (HD + 1)), F16, kind="ExternalInput")
    mk_in = nc.dram_tensor("mk", (NCX, NX), U8, kind="ExternalInput")
    wq_in = nc.dram_tensor("wq", (D, DL), F32R, kind="ExternalInput")
    wo_in = nc.dram_tensor("wo", (DL, D), F32R, kind="ExternalInput")
    lnw_in = nc.dram_tensor("lnw", (D,), F32, kind="ExternalInput")
    lnb_in = nc.dram_tensor("lnb", (D,), F32R, kind="ExternalInput")
    o_out = nc.dram_tensor("o", (NX, D), F32, kind="ExternalOutput")

    with TileContext(nc) as tc:
        with (
            tc.tile_pool(name="cst", bufs=1) as cst,
            tc.tile_pool(name="xsq", bufs=2) as xsqp,
            tc.tile_pool(name="mskf", bufs=3) as mskp,
            tc.tile_pool(name="pt", bufs=6) as ptp,
            tc.tile_pool(name="rb", bufs=2) as rbp,
            tc.tile_pool(name="osb", bufs=3) as osbp,
            tc.tile_pool(name="psA", bufs=2, space="PSUM") as psA,
            tc.tile_pool(name="psB", bufs=4, space="PSUM") as psB,
        ):
            ident = cst.tile([128, 128], F32)
            make_identity(nc, ident)
            ones_col = cst.tile([128, 1], F32R)
            nc.vector.memset(ones_col.bitcast(mybir.dt.uint32), 0x3F800000)
            ones_row = cst.tile([1, 128], F32)
            nc.vector.memset(ones_row, 1.0)
            ones_rowr = cst.tile([1, 64], F32R)
            nc.vector.memset(ones_rowr.bitcast(mybir.dt.uint32), 0x3F800000)
            # dummy Ln+Exp so the ACT table set loads during the input DMAs
            warm = cst.tile([1, 2], F32)
            nc.vector.memset(warm, 1.0)
            nc.scalar.activation(out=warm[:, 0:1], in_=warm[:, 0:1], func=Log)
            nc.scalar.activation(out=warm[:, 1:2], in_=warm[:, 1:2], func=Exp)

            lnw_t = cst.tile([128, 8], F32)
            nc.sync.dma_start(out=lnw_t, in_=lnw_in[:].rearrange("(t p) -> p t", p=128))
            lnb_t = cst.tile([128, 8], F32R)
            nc.sync.dma_start(out=lnb_t, in_=lnb_in[:].rearrange("(t p) -> p t", p=128))
            xT_t = cst.tile([128, 8, NX], F32R)
            xT_re = xT_in[:].rearrange("(t p) n -> p t n", p=128)
            for t in range(8):
                nc.sync.dma_start(out=xT_t[:, t], in_=xT_re[:, t])
            wq_t = cst.tile([128, 8, DL], F32R)
            wq_re = wq_in[:].rearrange("(t p) n -> p t n", p=128)
            for t in range(0, 8, 2):
                nc.sync.dma_start(out=wq_t[:, t:t + 2], in_=wq_re[:, t:t + 2])
            cT_t = cst.tile([128, 2, NCX], F32R)
            cT_re = cT_in[:].rearrange("(t p) n -> p t n", p=128)
            mk_t = cst.tile([128, NKC, NX], U8)
            mk_re = mk_in[:].rearrange("(t p) n -> p t n", p=128)
            kvN = cst.tile([128, NKC, HPC, HD + 1], F16)
            # first score chunk + first mask chunk, then the whole kv (one
            # flat DMA — ones column pre-padded on host), then the rest
            nc.sync.dma_start(out=cT_t[:, :, 0:256], in_=cT_re[:, :, 0:256])
            nc.sync.dma_start(out=cT_t[:, :, 256:512], in_=cT_re[:, :, 256:512])
            nc.sync.dma_start(out=mk_t[:, 0:16], in_=mk_re[:, 0:16])
            nc.sync.dma_start(
                out=kvN.rearrange("p t h c -> p (t h c)"), in_=cN_in[:]
            )
            for ch in range(1, 8):
                nc.sync.dma_start(
                    out=cT_t[:, :, ch * 512:(ch + 1) * 512],
                    in_=cT_re[:, :, ch * 512:(ch + 1) * 512],
                )
            nc.sync.dma_start(out=mk_t[:, 16:32], in_=mk_re[:, 16:32])
            # wo is only needed for the tail projection — load it last
            wo_t = cst.tile([128, 2, D], F32R)
            wo_re = wo_in[:].rearrange("(t p) n -> p t n", p=128)
            for t in range(2):
                nc.sync.dma_start(out=wo_t[:, t], in_=wo_re[:, t])

            # ---- LayerNorm stats from x^T: mean and E[x^2] per token ----
            ps_su = psA.tile([1, NX], F32, tag="A")
            ps_sq = psA.tile([1, NX], F32, tag="A")
            for t in range(8):
                xsq = xsqp.tile([128, NX], F32R)
                nc.vector.tensor_mul(xsq, xT_t[:, t], xT_t[:, t])
                nc.tensor.matmul(ps_su, ones_col, xT_t[:, t], start=(t == 0), stop=(t == 7))
                nc.tensor.matmul(ps_sq, ones_col, xsq, start=(t == 0), stop=(t == 7))
            mu = cst.tile([1, NX], F32)
            nc.scalar.mul(out=mu, in_=ps_su, mul=1.0 / D)
            ex2 = cst.tile([1, NX], F32)
            nc.scalar.mul(out=ex2, in_=ps_sq, mul=1.0 / D)
            var = cst.tile([1, NX], F32)
            nc.vector.tensor_mul(var, mu, mu)
            nc.vector.tensor_sub(var, ex2, var)
            eps_t = cst.tile([1, 1], F32)
            nc.vector.memset(eps_t, LN_EPS)
            lnv = cst.tile([1, NX], F32)
            nc.scalar.activation(out=lnv, in_=var, func=Log, bias=eps_t)
            rstd = cst.tile([1, NX], F32)
            nc.scalar.activation(out=rstd, in_=lnv, func=Exp, scale=-0.5)
            negmu = cst.tile([1, NX], F32)
            nc.vector.tensor_scalar_mul(negmu, mu, -1.0)

            # ---- bWq row = ln_b @ Wq (raw Wq), ln_w fold into wq2,
            # ---- wWq row = colsum(wq2); all independent of the stats chain ----
            wq2 = cst.tile([128, 8, DL], F32R)
            for t in range(8):
                nc.vector.tensor_scalar_mul(wq2[:, t], wq_t[:, t], lnw_t[:, t:t + 1])
            ps_bw = psA.tile([1, DL], F32, tag="A")
            for t in range(8):
                nc.tensor.matmul(
                    ps_bw,
                    lnb_t[:, t:t + 1].bitcast(F32),
                    wq_t[:, t].bitcast(F32),
                    start=(t == 0), stop=(t == 7),
                )
            bwq = cst.tile([1, DL], F32)
            nc.vector.tensor_copy(out=bwq, in_=ps_bw)
            ps_ww = psA.tile([1, DL], F32, tag="A")
            for t in range(8):
                nc.tensor.matmul(ps_ww, ones_col, wq2[:, t], start=(t == 0), stop=(t == 7))
            wwq = cst.tile([1, DL], F32)
            nc.vector.tensor_copy(out=wwq, in_=ps_ww)
            rsc_tok = cst.tile([1, NX], F32)
            nc.vector.reciprocal(out=rsc_tok, in_=rstd)

            ps_rb = psA.tile([128, NX], F32, tag="A")
            nc.tensor.matmul(ps_rb, ones_row, rstd, start=True, stop=True)
            rstd_bc = cst.tile([128, NX], F32R)
            nc.vector.tensor_copy(out=rstd_bc, in_=ps_rb)
            # ---- q^T projection: rstd * (Wq'^T x^T - wWq mu + bWq/rstd) ----
            qT = cst.tile([128, 2, NX], F32R)
            for dt in range(2):
                ps_q = psA.tile([128, NX], F32, tag="A")
                for t in range(8):
                    nc.tensor.matmul(
                        ps_q,
                        wq2[:, t, dt * 128:(dt + 1) * 128],
                        xT_t[:, t],
                        start=(t == 0), stop=False,
                    )
                nc.tensor.matmul(
                    ps_q, wwq[:, dt * 128:(dt + 1) * 128], negmu, start=False, stop=False
                )
                nc.tensor.matmul(
                    ps_q, bwq[:, dt * 128:(dt + 1) * 128], rsc_tok, start=False, stop=True
                )
                nc.vector.tensor_mul(qT[:, dt], ps_q, rstd_bc)

            # ---- natural-layout q (the appended-kv part) with ones column ----
            kvq = cst.tile([128, NKQ, HPC, HD + 1], F16)
            nc.gpsimd.memset(kvq[:, :, :, HD:HD + 1], 1.0)

            def build_kvq():
                for h in range(HPC):
                    pb = (h % 2) * 64
                    ps_t = psB.tile([128, NKQ * HD], F32, tag="B", name=f"tp{h}")
                    for tq in range(NKQ):
                        nc.tensor.transpose(
                            ps_t[:, tq * HD:(tq + 1) * HD],
                            qT[pb:pb + 64, h // 2, tq * 128:(tq + 1) * 128].bitcast(F32),
                            ident[pb:pb + 64, pb:pb + 64],
                        )
                    nc.vector.tensor_copy(
                        out=kvq[:, :, h, 0:HD],
                        in_=ps_t.rearrange("p (t d) -> p t d", d=HD),
                    )

            # ---- main attention: two passes, one head-pair each, so pass 0's
            # ---- normalize + dt=0 projection overlap pass 1's attention ----
            ps_aT = [psB.tile([HD + 1, NX], F32, tag="B", name=f"aT{i}") for i in range(HPC)]
            aTs = cst.tile([128, 2, NX], F32R)
            o0 = cst.tile([128, 4, 2, 512], F32)

            def scores_tile(kt, hp):
                ps_s = psA.tile([128, 2 * NX], F32, tag="A", name=f"s{hp}")
                for hh in range(2):
                    h = 2 * hp + hh
                    pb = (h % 2) * 64
                    if kt < NKQ:
                        lhsT = qT[pb:pb + 64, h // 2, kt * 128:(kt + 1) * 128]
                    else:
                        c0 = (kt - NKQ) * 128
                        lhsT = cT_t[pb:pb + 64, h // 2, c0:c0 + 128]
                    nc.tensor.matmul(
                        ps_s[:, hh * NX:(hh + 1) * NX],
                        lhsT,
                        qT[pb:pb + 64, h // 2, :],
                        start=True, stop=True,
                    )
                pt = ptp.tile([128, 2 * NX], F16, name=f"pt{hp}")
                nc.scalar.activation(out=pt, in_=ps_s, func=Exp, scale=SCALE)
                return pt

            def v_tile(kt, hp, pt):
                for hh in range(2):
                    h = 2 * hp + hh
                    if kt < NKQ:
                        lhsTv = kvq[:, kt, h, :]
                    else:
                        lhsTv = kvN[:, kt - NKQ, h, :]
                    nc.tensor.matmul(
                        ps_aT[h],
                        lhsTv,
                        pt[:, hh * NX:(hh + 1) * NX],
                        start=(kt == 0), stop=(kt == NKT - 1),
                    )

            def attn_tile(kt, pairs, tag):
                # one masked key tile for the given head pairs
                mf = mskp.tile([128, NX], F16, name=f"mf{tag}")
                nc.gpsimd.tensor_copy(out=mf, in_=mk_t[:, kt - NKQ])
                for hp in pairs:
                    pt = scores_tile(kt, hp)
                    ptv = pt.rearrange("p (a b) -> p a b", a=2)
                    nc.vector.tensor_mul(ptv, ptv, mf.rearrange("p (a b) -> p a b", a=1).broadcast_to((128, 2, NX)))
                    v_tile(kt, hp, pt)

            def normalize_pair(hp):
                # pair 1 normalizes in the tail when ACT is idle: evacuate its
                # denominator broadcasts on ACT instead of the busy DVE FIFO
                rb_copy = nc.vector.tensor_copy if hp == 0 else (
                    lambda out, in_: nc.scalar.copy(out=out, in_=in_))
                # per-head softmax division at aT evacuation; emit both heads'
                # recips first so neither blocks the other behind the DVE FIFO
                rcs, pss = [], []
                with nc.allow_low_precision(reason="fp32r recip feeds fp32r bcast matmul"):
                    for hh in range(2):
                        h = 2 * hp + hh
                        rc = rbp.tile([1, NX], F32R, tag="rc", name=f"rc{h}")
                        nc.vector.reciprocal(out=rc, in_=ps_aT[h][HD:HD + 1, :])
                        rcs.append(rc)
                for hh in range(2):
                    h = 2 * hp + hh
                    ps_rc = psA.tile([64, NX], F32, tag="A", name=f"rcb{h}")
                    nc.tensor.matmul(ps_rc, ones_rowr, rcs[hh], start=True, stop=True)
                    pss.append(ps_rc)
                for hh in range(2):
                    h = 2 * hp + hh
                    pb = (h % 2) * 64
                    rb = rbp.tile([64, NX], F32, tag="rb", name=f"rb{h}")
                    rb_copy(out=rb, in_=pss[hh])
                    nc.vector.tensor_mul(aTs[pb:pb + 64, h // 2, :], ps_aT[h][0:HD, :], rb)

            SPLIT = 30
            # q-region: scores+exp first (they only need qT), kvq transposes
            # overlap with the exps, then the q-region V matmuls
            q_pts = [(kt, hp, scores_tile(kt, hp)) for hp in range(2) for kt in range(NKQ)]
            build_kvq()
            for kt, hp, pt in q_pts:
                v_tile(kt, hp, pt)
            # phase 1: all four heads while inputs stream in
            for kt in range(NKQ, SPLIT):
                attn_tile(kt, (0, 1), "a")
            # phase 2: finish pair 0
            for kt in range(SPLIT, NKT):
                attn_tile(kt, (0,), "b")
            for kt in range(SPLIT, SPLIT + 3):
                attn_tile(kt, (1,), "c")
            normalize_pair(0)
            # dt=0 half of the output projection overlaps phase 3
            for mt in range(4):
                for nt in range(2):
                    ps_o = psB.tile([128, 512], F32, tag="B", name="o0ps")
                    nc.tensor.matmul(
                        ps_o,
                        aTs[:, 0, mt * 128:(mt + 1) * 128],
                        wo_t[:, 0, nt * 512:(nt + 1) * 512],
                        start=True, stop=True,
                    )
                    nc.vector.tensor_copy(out=o0[:, mt, nt], in_=ps_o)
            # phase 3: finish pair 1
            for kt in range(SPLIT + 3, NKT):
                attn_tile(kt, (1,), "c")
            normalize_pair(1)
            # tail: dt=1 matmul, add the dt=0 partial at evacuation, DMA out
            for mt in range(4):
                ob = osbp.tile([128, 2, 512], F32)
                for nt in range(2):
                    ps_o = psB.tile([128, 512], F32, tag="B", name="o1ps")
                    nc.tensor.matmul(
                        ps_o,
                        aTs[:, 1, mt * 128:(mt + 1) * 128],
                        wo_t[:, 1, nt * 512:(nt + 1) * 512],
                        start=True, stop=True,
                    )
                    nc.vector.tensor_add(ob[:, nt], ps_o, o0[:, mt, nt])
                nc.sync.dma_start(
                    out=o_out[mt * 128:(mt + 1) * 128, :],
                    in_=ob.rearrange("p a b -> p (a b)"),
                )

    nc.compile()
    return nc


def _get_prog():
    global _PROG
    if _PROG is None:
        _PROG = _build_program()
    return _PROG


def kernel(x, c, attn_mask, ln_w, ln_b, Wq, Wo):
    global LAST_RESULT
    from concourse.bass_utils import run_bass_kernel_spmd

    x = np.asarray(x, dtype=np.float32)
    c = np.asarray(c, dtype=np.float32)
    mask = np.asarray(attn_mask)
    ln_w = np.asarray(ln_w, dtype=np.float32)
    ln_b = np.asarray(ln_b, dtype=np.float32)
    Wq = np.asarray(Wq, dtype=np.float32)
    Wo = np.asarray(Wo, dtype=np.float32)

    nc = _get_prog()

    in_maps = []
    for b in range(B):
        xTb = np.ascontiguousarray(x[b].T)
        cTb = np.ascontiguousarray(c[b].T)
        mkb = np.ascontiguousarray(mask[b].T).astype(np.uint8)
        for g in range(H // HPC):
            sl = slice(g * DL, (g + 1) * DL)
            cNp = np.ones((128, NKC, HPC, HD + 1), dtype=np.float16)
            cNp[:, :, :, :HD] = (
                c[b][:, sl]
                .reshape(NKC, 128, HPC, HD)
                .transpose(1, 0, 2, 3)
                .astype(np.float16)
            )
            in_maps.append({
                "xT": xTb,
                "cT": np.ascontiguousarray(cTb[sl]),
                "cN": cNp.reshape(128, NKC * HPC * (HD + 1)),
                "mk": mkb,
                "wq": np.ascontiguousarray(Wq[:, sl]),
                "wo": np.ascontiguousarray(Wo[sl, :]),
                "lnw": ln_w,
                "lnb": ln_b,
            })

    res = run_bass_kernel_spmd(nc, in_maps, core_ids=list(range(8)))
    LAST_RESULT = res

    o = np.zeros((B, NX, D), dtype=np.float32)
    for b in range(B):
        for g in range(H // HPC):
            o[b] += res.results[b * (H // HPC) + g]["o"]

    kv0 = np.ascontiguousarray(
        c.reshape(B, NCX, H, HD).transpose(0, 2, 1, 3)
    )
    return o, kv0
